# revision 19
# baseline (speedup 1.0000x reference)
"""Multi-head attention (B=2, S=2048, D=1024, H=16) on 8 trn2 NeuronCores.

Sharding: 2-way over batch x 4-way over head groups (4 heads / 256 cols per
core). No cross-core communication.

Per-core kernel (Tile):
  prefix:  load X_k, X_v and the first X_q block (cast f32->bf16 during DMA),
           transpose 128x128 chunks via regular matmul against identity
           (counts as PE activity so the HAM clock gate stays open), project
           kT [256, 2048] (head dim on partitions), v natural [2048, 256]
           stored as [v | 1] per ki-tile (the ones column makes the AV matmul
           also emit softmax row-sums), and qT for block 0.
  stream:  per (head-pair, qi-block of 512): for each ki-tile: S^T = kT.T@qT
           with the two heads row-packed on the PE (K=64 each) into one
           [128, 1024] psum tile (bufs=2), one exp ACTIVATE per ki-tile
           (scale=1/8 folded in), AV matmuls one ki-tile behind the scores so
           the exp stream never stalls. The remaining X_q blocks are loaded /
           transposed / projected in small slices woven into the first three
           units' ki-loops (PSUM slots shared with the AV pool). Unit
           epilogues (out^T -> out transpose + softmax normalize) are split
           in half and woven into the next unit's ki-loop the same way.
"""

import os
import sys

import numpy as np

import concourse.bass as bass
import concourse.tile as tile
from concourse import bacc, mybir
from concourse.masks import make_identity

B, S, D = 2, 2048, 1024
H, HD = 16, 64
N_CORES = 8
GROUPS = 4  # head groups (cores per batch)
NH = H // GROUPS  # local heads per core = 4
C = NH * HD  # local output cols = 256
P = 128
DB = D // P  # 8 d-chunks
CB = C // P  # 2 c-chunks (head pairs)

f32 = mybir.dt.float32
bf16 = mybir.dt.bfloat16
f32r = mybir.dt.float32r

AF = mybir.ActivationFunctionType


def _install_ntff_hook_shim():
    """Best-effort: register the axon NTFF profile hook so a traced run
    (e.g. BASS_TRACE=1) works even when the image's antenv lacks axon_hooks."""
    try:
        import antenv.axon_hooks  # noqa: F401

        return
    except ImportError:
        pass
    try:
        import types

        _hook = [None]
        mod = types.ModuleType("antenv.axon_hooks")
        mod.set_axon_ntff_profile_hook = lambda h: _hook.__setitem__(0, h)
        mod.get_axon_ntff_profile_hook = lambda: _hook[0]
        sys.modules["antenv.axon_hooks"] = mod
        from trn_agent_boot.trn_boot import _ntff_profile_via_ctypes

        so = "/opt/axon/libaxon_pjrt.so"
        if os.path.exists(so):
            mod.set_axon_ntff_profile_hook(_ntff_profile_via_ctypes(so))
    except Exception:
        pass


def build_nc(dt_mode: str = "fp16", s: int = S):
    """Trace + compile the per-core Bass kernel. dt_mode in {"fp16", "bf16", "f32r"}."""
    assert s % 512 == 0
    SB = s // P  # ki-tiles
    NBLK = s // 512  # 512-row s-blocks
    QB = 512  # qi-block
    NQB = s // QB
    NJ = QB // P  # 128-chunks per qi-block = 4
    # overlap q blocks 1.. with the attention stream only at full size
    OVERLAP_Q = SB >= 16 and NBLK == 4

    if dt_mode == "bf16":
        dt_x = bf16  # storage dtype of matmul inputs

        def mm(ap):
            return ap
    elif dt_mode == "fp16":
        dt_x = mybir.dt.float16

        def mm(ap):
            return ap
    else:
        dt_x = f32

        def mm(ap):
            return ap.bitcast(f32r)

    nc = bacc.Bacc(
        "TRN2", target_bir_lowering=False, debug=False, num_devices=N_CORES
    )

    xq = nc.dram_tensor("xq", [s, D], f32, kind="ExternalInput").ap()
    xk = nc.dram_tensor("xk", [s, D], f32, kind="ExternalInput").ap()
    xv = nc.dram_tensor("xv", [s, D], f32, kind="ExternalInput").ap()
    wq = nc.dram_tensor("wq", [D, C], f32, kind="ExternalInput").ap()
    wk = nc.dram_tensor("wk", [D, C], f32, kind="ExternalInput").ap()
    wv = nc.dram_tensor("wv", [D, C], f32, kind="ExternalInput").ap()
    bq = nc.dram_tensor("bq", [C], f32, kind="ExternalInput").ap()
    bk = nc.dram_tensor("bk", [C], f32, kind="ExternalInput").ap()
    bv = nc.dram_tensor("bv", [C], f32, kind="ExternalInput").ap()
    out = nc.dram_tensor("out", [s, C], f32, kind="ExternalOutput").ap()

    with tile.TileContext(nc) as tc:
        with (
            tc.tile_pool(name="const", bufs=1) as const_pool,
            tc.tile_pool(name="wts", bufs=1) as wts_pool,
            tc.tile_pool(name="qkv", bufs=1) as qkv_pool,
            tc.tile_pool(name="xn", bufs=4) as xn_pool,
            tc.tile_pool(name="xt", bufs=3) as xt_pool,
        ):
            ident = const_pool.tile([P, P], dt_x)
            make_identity(nc, ident[:])
            ident_f = const_pool.tile([P, P], f32)
            make_identity(nc, ident_f[:])

            # First x block's DMA goes on the queue before the weights so the
            # PE can start transposing ~6us in; weights follow (k first).
            xn_first = xn_pool.tile([P, 4, D], dt_x, tag="xn", name="xn_first")
            for t in range(4):
                nc.gpsimd.dma_start(
                    xn_first[:, t, :],
                    xk[t * P : (t + 1) * P, :].rearrange("(t p) d -> p t d", p=P)[
                        :, 0
                    ],
                )

            # weights: [p, dc, c] where d = dc*128 + p
            w_sb = {}
            for name, ap in (("k", wk), ("v", wv), ("q", wq)):
                t = wts_pool.tile([P, DB, C], dt_x, tag=f"w_{name}", name=f"w_{name}")
                nc.gpsimd.dma_start(t[:], ap.rearrange("(dc p) c -> p dc c", p=P))
                w_sb[name] = t
            # biases for q/k: [p, cc] with c = cc*128 + p
            b_sb = {}
            for name, ap in (("q", bq), ("k", bk)):
                t = const_pool.tile([P, CB], f32, tag=f"b_{name}", name=f"b_{name}")
                nc.sync.dma_start(t[:], ap.rearrange("(cc p) -> p cc", p=P))
                b_sb[name] = t
            # v bias as a row vector + ones row for the K=1 bias matmul
            bv_row = const_pool.tile([1, C], dt_x)
            nc.gpsimd.dma_start(bv_row[:], bv[None, :])
            ones_row = const_pool.tile([1, P], dt_x)
            nc.vector.memset(ones_row[:], 1.0)

            # projection outputs (persistent)
            qT = qkv_pool.tile([P, CB, s], dt_x)  # q^T: [c%128, c//128, s]
            kT = qkv_pool.tile([P, CB, s], dt_x)
            v1 = qkv_pool.tile([P, SB, NH, HD + 1], dt_x)  # [ki%128, ki//128, h, d|1]
            nc.vector.memset(v1[:, :, :, HD : HD + 1], 1.0)

            def emit_xn_dma(x_ap, blk):
                xn = xn_pool.tile([P, 4, D], dt_x, tag="xn")
                src = x_ap[blk * 512 : (blk + 1) * 512, :].rearrange(
                    "(t p) d -> p t d", p=P
                )
                for t in range(4):
                    nc.gpsimd.dma_start(xn[:, t, :], src[:, t, :])
                return xn

            def emit_qk_proj_cc(name, blk, xt, cc, pj_tile):
                dsttile = qT if name == "q" else kT
                for dc in range(DB):
                    nc.tensor.matmul(
                        pj_tile[:],
                        mm(w_sb[name][:, dc, cc * P : (cc + 1) * P]),
                        mm(xt[:, dc, :]),
                        start=(dc == 0),
                        stop=(dc == DB - 1),
                    )
                nc.vector.tensor_scalar_add(
                    dsttile[:, cc, blk * 512 : (blk + 1) * 512],
                    pj_tile[:],
                    b_sb[name][:, cc : cc + 1],
                )

            # ---------------- prefix: k, v, q-block-0 ----------------
            with (
                tc.tile_pool(name="ps_tr", bufs=2, space="PSUM") as ps_tr,
                tc.tile_pool(name="ps_pj", bufs=2, space="PSUM") as ps_pj,
                tc.tile_pool(name="ps_pv", bufs=2, space="PSUM") as ps_pv,
            ):
                n_evict = 0

                def emit_proj(name, blk, xt):
                    if name in ("q", "k"):
                        for cc in range(CB):
                            ps = ps_pj.tile([P, 512], f32, tag="pj")
                            emit_qk_proj_cc(name, blk, xt, cc, ps)
                    else:
                        for t in range(4):
                            sc = blk * 4 + t
                            ps = ps_pv.tile([P, C], f32, tag="pv")
                            for dc in range(DB):
                                nc.tensor.matmul(
                                    ps[:],
                                    mm(xt[:, dc, t * P : (t + 1) * P]),
                                    mm(w_sb["v"][:, dc, :]),
                                    start=(dc == 0),
                                    stop=False,
                                )
                            nc.tensor.matmul(
                                ps[:],
                                mm(ones_row[:, :]),
                                mm(bv_row[:, :]),
                                start=False,
                                stop=True,
                            )
                            nc.vector.tensor_copy(
                                v1[:, sc, :, 0:HD],
                                ps.rearrange("p (h e) -> p h e", h=NH),
                            )

                prefix_items = [("k", xk, blk) for blk in range(NBLK)]
                prefix_items += [("v", xv, blk) for blk in range(NBLK)]
                prefix_items += [
                    ("q", xq, blk) for blk in range(1 if OVERLAP_Q else NBLK)
                ]
                pending = None  # (name, blk, xt) with projections still to emit
                for name, x_ap, blk in prefix_items:
                    if name == "k" and blk == 0:
                        xn = xn_first
                    else:
                        xn = emit_xn_dma(x_ap, blk)
                    xt = xt_pool.tile([P, DB, 512], dt_x, tag="xt")
                    for t in range(4):
                        # 8 transposed chunks into one [128, 8, 128] psum
                        # tile, evicted with a single wide copy.
                        ps = ps_tr.tile([P, DB, P], f32, tag="tr")
                        for dc in range(DB):
                            nc.tensor.matmul(
                                ps[:, dc, :],
                                mm(xn[:, t, dc * P : (dc + 1) * P]),
                                mm(ident[:]),
                                start=True,
                                stop=True,
                            )
                        dst = xt.rearrange("p dc (t q) -> p t dc q", q=P)[:, t]
                        nc.vector.tensor_copy(dst, ps[:])
                        n_evict += 1
                    if pending is not None:
                        emit_proj(*pending)
                    pending = (name, blk, xt)
                emit_proj(*pending)

            # ---------------- attention stream ----------------
            with (
                tc.tile_pool(name="ps_sc", bufs=2, space="PSUM") as ps_sc,
                tc.tile_pool(name="ps_av", bufs=4, space="PSUM") as ps_av,
                tc.tile_pool(name="pexp", bufs=3) as p_pool,
                tc.tile_pool(name="osb", bufs=2) as o_pool,
                tc.tile_pool(name="outsb", bufs=2) as out_pool,
            ):

                avpack = os.environ.get("MHA_AVPACK", "1") == "1"

                def emit_av(hp, av, pex, ktp, last):
                    if not avpack:
                        for head in range(2):
                            nc.tensor.matmul(
                                av[head][:],
                                mm(v1[:, ktp, 2 * hp + head, :]),
                                mm(pex[:, head * QB : (head + 1) * QB]),
                                start=(ktp == 0),
                                stop=last,
                            )
                        return
                    # The K=128 AV contraction is split into two K=64 halves on
                    # distinct PE row groups (tile_position (0,0)/(64,0), auto-
                    # inferred from base partitions). Pairs (h0,half0)+(h1,half1)
                    # and (h0,half1)+(h1,half0) run concurrently on the array,
                    # halving the AV stream cost.
                    order = (
                        ((0, 0), (1, 1), (0, 1), (1, 0))
                        if os.environ.get("MHA_AVORD", "0") == "0"
                        else ((0, 0), (0, 1), (1, 1), (1, 0))
                    )
                    first = {}
                    lastmm = {}
                    for head, half in order:
                        first.setdefault(head, (head, half))
                        lastmm[head] = (head, half)
                    for head, half in order:
                        r0 = half * 64
                        nc.tensor.matmul(
                            av[head][:],
                            mm(v1[r0 : r0 + 64, ktp, 2 * hp + head, :]),
                            mm(pex[r0 : r0 + 64, head * QB : (head + 1) * QB]),
                            start=(ktp == 0 and first[head] == (head, half)),
                            stop=(last and lastmm[head] == (head, half)),
                        )

                def emit_tail_half(hp, qb, av, head, out_sb):
                    o_sb = o_pool.tile(
                        [HD + 1, QB], f32, tag="osb", name=f"osb{hp}_{qb}_{head}"
                    )
                    nc.vector.tensor_copy(o_sb[:], av[head][:])
                    tp = ps_av.tile(
                        [P, NJ, HD + 1],
                        f32,
                        tag="av",
                        name=f"tp{hp}_{qb}_{head}",
                    )
                    for j in range(NJ):
                        nc.tensor.transpose(
                            tp[:, j, :],
                            o_sb[:, j * P : (j + 1) * P],
                            ident_f[: HD + 1, : HD + 1],
                        )
                    rsb = o_pool.tile(
                        [P, NJ], f32, tag="rsb", name=f"rsb{hp}_{qb}_{head}"
                    )
                    nc.vector.reciprocal(rsb[:], tp[:, :, HD])
                    for j in range(NJ):
                        nc.vector.tensor_scalar_mul(
                            out_sb[:, j, head * HD : (head + 1) * HD],
                            tp[:, j, 0:HD],
                            rsb[:, j : j + 1],
                        )

                def emit_tail_dma(hp, qb, out_sb):
                    q0 = qb * QB
                    nc.sync.dma_start(
                        out[q0 : q0 + QB, hp * P : (hp + 1) * P].rearrange(
                            "(j p) c -> p j c", p=P
                        ),
                        out_sb[:],
                    )

                # woven q-block work: unit index -> q block to process
                qwork = {}
                if OVERLAP_Q:
                    for u, blk in enumerate(range(1, NBLK)):
                        qwork[u] = blk
                qstate = {}  # per live q block: dict(xn=, xt=, pj=)

                def emit_qwork(blk, kt):
                    st = qstate[blk]
                    if kt == 0:
                        st["xn"] = emit_xn_dma(xq, blk)
                        st["xt"] = xt_pool.tile(
                            [P, DB, 512], dt_x, tag="xt", name=f"xt_q{blk}"
                        )
                    elif 3 <= kt <= 6:
                        t = kt - 3
                        for dhalf in range(2):
                            tr = ps_av.tile(
                                [P, 4, P],
                                f32,
                                tag="av",
                                name=f"tr_q{blk}_{t}_{dhalf}",
                            )
                            for i in range(4):
                                dc = dhalf * 4 + i
                                nc.tensor.matmul(
                                    tr[:, i, :],
                                    mm(st["xn"][:, t, dc * P : (dc + 1) * P]),
                                    mm(ident[:]),
                                    start=True,
                                    stop=True,
                                )
                            nc.vector.tensor_copy(
                                st["xt"][
                                    :, dhalf * 4 : dhalf * 4 + 4, t * P : (t + 1) * P
                                ],
                                tr[:],
                            )
                    elif 7 <= kt <= 14:
                        cc, half = divmod(kt - 7, 4)
                        if half == 0:
                            st["pj"] = ps_av.tile(
                                [P, 512], f32, tag="av", name=f"pj_q{blk}_{cc}"
                            )
                        for dc in range(half * 2, half * 2 + 2):
                            nc.tensor.matmul(
                                st["pj"][:],
                                mm(w_sb["q"][:, dc, cc * P : (cc + 1) * P]),
                                mm(st["xt"][:, dc, :]),
                                start=(dc == 0),
                                stop=(dc == DB - 1),
                            )
                        if half == 3:
                            nc.vector.tensor_scalar_add(
                                qT[:, cc, blk * 512 : (blk + 1) * 512],
                                st["pj"][:],
                                b_sb["q"][:, cc : cc + 1],
                            )
                            del st["pj"]

                KT_A = max(1, SB // 8)
                KT_B = max(KT_A + 1, min(4, SB - 1))
                tail_prev = None  # (hp, qb, av) of the finished unit
                tail_outsb = None
                uidx = 0
                for hp in range(CB):  # head pair (c-chunk)
                    for qb in range(NQB):  # qi block of 512
                        q0 = qb * QB
                        if uidx in qwork:
                            qstate[qwork[uidx]] = {}
                        av = {}
                        for head in range(2):
                            av[head] = ps_av.tile(
                                [HD + 1, QB], f32, tag="av", name=f"av{hp}_{qb}_{head}"
                            )
                        # scores/exp stream one ki-tile ahead of the AV
                        # matmuls so the ACT exp stream never stalls on PE.
                        pex_q = []
                        for kt in range(SB):
                            sc_ps = ps_sc.tile([P, 2 * QB], f32, tag="sc")
                            for head in range(2):
                                r0 = head * HD
                                nc.tensor.matmul(
                                    sc_ps[:, head * QB : (head + 1) * QB],
                                    mm(kT[r0 : r0 + HD, hp, kt * P : (kt + 1) * P]),
                                    mm(qT[r0 : r0 + HD, hp, q0 : q0 + QB]),
                                    start=True,
                                    stop=True,
                                )
                            pex = p_pool.tile([P, 2 * QB], dt_x, tag="pex")
                            nc.scalar.activation(
                                pex[:], sc_ps[:], AF.Exp, bias=0.0, scale=0.125
                            )
                            pex_q.append(pex)
                            if kt >= 1:
                                emit_av(hp, av, pex_q[kt - 1], kt - 1, False)
                            if kt == KT_A and tail_prev is not None:
                                tail_outsb = out_pool.tile(
                                    [P, NJ, P],
                                    f32,
                                    tag="outsb",
                                    name=f"outsb{tail_prev[0]}_{tail_prev[1]}",
                                )
                                emit_tail_half(*tail_prev, 0, tail_outsb)
                            if kt == KT_B and tail_prev is not None:
                                emit_tail_half(*tail_prev, 1, tail_outsb)
                                emit_tail_dma(tail_prev[0], tail_prev[1], tail_outsb)
                                tail_prev = None
                            if uidx in qwork:
                                emit_qwork(qwork[uidx], kt)
                        emit_av(hp, av, pex_q[SB - 1], SB - 1, True)
                        tail_prev = (hp, qb, av)
                        uidx += 1
                tail_outsb = out_pool.tile(
                    [P, NJ, P], f32, tag="outsb", name="outsb_last"
                )
                emit_tail_half(*tail_prev, 0, tail_outsb)
                emit_tail_half(*tail_prev, 1, tail_outsb)
                emit_tail_dma(tail_prev[0], tail_prev[1], tail_outsb)
    nc.compile()
    return nc


def build_nc_v3(dt_mode: str = "fp16", s: int = S):
    """Sweep-structured kernel: kt-block-outer so the softmax exp stream (the
    ScalarE wall, ~147us) starts ~16us in and never starves.

    Stream = NBLK sweeps x NU units x KB kt-tiles. AV partials accumulate in
    PSUM within a sweep-visit and are folded into an SBUF accumulator between
    sweeps. All input-block production (DMA, PE transposes, projections) except
    (k0, q0) is woven into the stream's PE slack via a deadline-forced work
    queue. PSUM: 4 banks scores (double-buffered) + 2 AV + 1 transpose + 1
    projection.
    """
    assert s % 512 == 0
    SB = s // P
    NBLK = s // 512
    KB = SB // NBLK  # 4 kt per sweep visit
    QB = 512
    NQB = s // QB
    NU = NQB * CB  # units: u -> (qb, hp)
    NJ = QB // P

    if dt_mode == "bf16":
        dt_x = bf16

        def mm(ap):
            return ap
    elif dt_mode == "fp16":
        dt_x = mybir.dt.float16

        def mm(ap):
            return ap
    else:
        dt_x = f32

        def mm(ap):
            return ap.bitcast(f32r)

    nc = bacc.Bacc(
        "TRN2", target_bir_lowering=False, debug=False, num_devices=N_CORES
    )

    xq = nc.dram_tensor("xq", [s, D], f32, kind="ExternalInput").ap()
    xk = nc.dram_tensor("xk", [s, D], f32, kind="ExternalInput").ap()
    xv = nc.dram_tensor("xv", [s, D], f32, kind="ExternalInput").ap()
    wq = nc.dram_tensor("wq", [D, C], f32, kind="ExternalInput").ap()
    wk = nc.dram_tensor("wk", [D, C], f32, kind="ExternalInput").ap()
    wv = nc.dram_tensor("wv", [D, C], f32, kind="ExternalInput").ap()
    bq = nc.dram_tensor("bq", [C], f32, kind="ExternalInput").ap()
    bk = nc.dram_tensor("bk", [C], f32, kind="ExternalInput").ap()
    bv = nc.dram_tensor("bv", [C], f32, kind="ExternalInput").ap()
    out = nc.dram_tensor("out", [s, C], f32, kind="ExternalOutput").ap()
    x_aps = {"q": xq, "k": xk, "v": xv}
    w_aps = {"q": wq, "k": wk, "v": wv}

    with tile.TileContext(nc) as tc:
        with (
            tc.tile_pool(name="const", bufs=1) as const_pool,
            tc.tile_pool(name="wts", bufs=1) as wts_pool,
            tc.tile_pool(name="qkv", bufs=1) as qkv_pool,
            tc.tile_pool(name="xn", bufs=4) as xn_pool,
            tc.tile_pool(name="xt", bufs=2) as xt_pool,
            tc.tile_pool(name="pex", bufs=6) as pex_pool,
            tc.tile_pool(name="osb", bufs=2) as o_pool,
            tc.tile_pool(name="outsb", bufs=2) as out_pool,
            tc.tile_pool(name="ps_sc", bufs=2, space="PSUM") as ps_sc,
            tc.tile_pool(name="ps_av", bufs=2, space="PSUM") as ps_av,
            tc.tile_pool(name="ps_tr", bufs=1, space="PSUM") as ps_tr,
            tc.tile_pool(name="ps_pj", bufs=1, space="PSUM") as ps_pj,
        ):
            ident = const_pool.tile([P, P], dt_x)
            make_identity(nc, ident[:])
            ident_f = const_pool.tile([P, P], f32)
            make_identity(nc, ident_f[:])
            ones_row = const_pool.tile([1, P], dt_x)
            nc.vector.memset(ones_row[:], 1.0)

            qT = qkv_pool.tile([P, CB, s], dt_x)
            kT = qkv_pool.tile([P, CB, s], dt_x)
            v1 = qkv_pool.tile([P, SB, NH, HD + 1], dt_x)
            nc.vector.memset(v1[:, :, :, HD : HD + 1], 1.0)
            o_acc = None
            if NBLK > 1:
                o_acc = qkv_pool.tile([HD + 1, NU, 2, QB], f32, name="o_acc")

            w_sb = {}
            b_sb = {}
            bv_row = const_pool.tile([1, C], dt_x)
            bstate = {}

            def emit_dma(name, blk):
                xn = xn_pool.tile([P, 4, D], dt_x, tag="xn", name=f"xn_{name}{blk}")
                src = x_aps[name][blk * 512 : (blk + 1) * 512, :].rearrange(
                    "(t p) d -> p t d", p=P
                )
                for t in range(4):
                    nc.gpsimd.dma_start(xn[:, t, :], src[:, t, :])
                bstate[(name, blk)]["xn"] = xn

            def emit_w(name):
                t = wts_pool.tile([P, DB, C], dt_x, tag=f"w_{name}", name=f"w_{name}")
                nc.gpsimd.dma_start(
                    t[:], w_aps[name].rearrange("(dc p) c -> p dc c", p=P)
                )
                w_sb[name] = t

            def emit_tr(name, blk, t, half):
                st = bstate[(name, blk)]
                if "xt" not in st:
                    st["xt"] = xt_pool.tile(
                        [P, DB, 512], dt_x, tag="xt", name=f"xt_{name}{blk}"
                    )
                ps = ps_tr.tile([P, 4, P], f32, tag="tr")
                for i in range(4):
                    dc = half * 4 + i
                    nc.tensor.matmul(
                        ps[:, i, :],
                        mm(st["xn"][:, t, dc * P : (dc + 1) * P]),
                        mm(ident[:]),
                        start=True,
                        stop=True,
                    )
                nc.vector.tensor_copy(
                    st["xt"][:, half * 4 : half * 4 + 4, t * P : (t + 1) * P],
                    ps[:],
                )

            def emit_pj(name, blk, cc):
                st = bstate[(name, blk)]
                dsttile = qT if name == "q" else kT
                ps = ps_pj.tile([P, 512], f32, tag="pj")
                for dc in range(DB):
                    nc.tensor.matmul(
                        ps[:],
                        mm(w_sb[name][:, dc, cc * P : (cc + 1) * P]),
                        mm(st["xt"][:, dc, :]),
                        start=(dc == 0),
                        stop=(dc == DB - 1),
                    )
                nc.vector.tensor_scalar_add(
                    dsttile[:, cc, blk * 512 : (blk + 1) * 512],
                    ps[:],
                    b_sb[name][:, cc : cc + 1],
                )

            def emit_pv(blk, t):
                st = bstate[("v", blk)]
                sc = blk * 4 + t
                ps = ps_pj.tile([P, 512], f32, tag="pj")
                for dc in range(DB):
                    nc.tensor.matmul(
                        ps[:, 0:C],
                        mm(st["xt"][:, dc, t * P : (t + 1) * P]),
                        mm(w_sb["v"][:, dc, :]),
                        start=(dc == 0),
                        stop=False,
                    )
                nc.tensor.matmul(
                    ps[:, 0:C],
                    mm(ones_row[:, :]),
                    mm(bv_row[:, :]),
                    start=False,
                    stop=True,
                )
                nc.vector.tensor_copy(
                    v1[:, sc, :, 0:HD],
                    ps[:, 0:C].rearrange("p (h e) -> p h e", h=NH),
                )

            def block_items(name, blk):
                items = []
                for t in range(4):
                    for half in range(2):
                        items.append(
                            (0.45, (lambda n, b, tt, hh: lambda: emit_tr(n, b, tt, hh))(name, blk, t, half))
                        )
                if name in ("q", "k"):
                    for cc in range(CB):
                        items.append(
                            (1.75, (lambda n, b, c: lambda: emit_pj(n, b, c))(name, blk, cc))
                        )
                else:
                    for t in range(4):
                        items.append(
                            (1.0, (lambda b, tt: lambda: emit_pv(b, tt))(blk, t))
                        )
                return items

            # ---------------- prefix ----------------
            for (name, blk) in [(n, b) for n in ("q", "k", "v") for b in range(NBLK)]:
                bstate[(name, blk)] = {}
            emit_dma("k", 0)
            emit_w("k")
            emit_dma("q", 0)
            emit_w("q")
            emit_dma("v", 0)
            emit_w("v")
            nc.gpsimd.dma_start(bv_row[:], bv[None, :])
            for name, ap in (("q", bq), ("k", bk)):
                t = const_pool.tile([P, CB], f32, tag=f"b_{name}", name=f"b_{name}")
                nc.sync.dma_start(t[:], ap.rearrange("(cc p) -> p cc", p=P))
                b_sb[name] = t
            for cost, fn in block_items("k", 0) + block_items("q", 0):
                fn()

            # ---------------- weave queue ----------------
            queue_blocks = [("v", 0)]
            queue_blocks += [("q", b) for b in range(1, NQB)]
            for b in range(1, NBLK):
                queue_blocks += [("k", b), ("v", b)]
            qitems = {key: block_items(*key) for key in queue_blocks}
            # DMA for block i leads by one queue position
            for i, key in enumerate(queue_blocks):
                lead = queue_blocks[max(0, i - 1)]
                dma_fn = (lambda k: lambda: emit_dma(*k))(key)
                qitems[lead].insert(0, (0.15, dma_fn))
            qlist = [(key, cost, fn) for key in queue_blocks for cost, fn in qitems[key]]
            qpos = [0]  # next index into qlist
            total_cost = sum(c for _, c, _ in qlist)
            done_upto = {}
            for i, (key, _, _) in enumerate(qlist):
                done_upto[key] = i + 1  # drain-through index per block

            def drain_through(key):
                tgt = done_upto.get(key, 0)
                while qpos[0] < tgt:
                    _, _, fn = qlist[qpos[0]]
                    fn()
                    qpos[0] += 1

            cum = [0.0]

            def budget_pop(slot, n_slots, drain_slots):
                tgt = total_cost * min(1.0, (slot + 1) / max(1, drain_slots))
                while qpos[0] < len(qlist) and cum[0] < tgt:
                    _, c, fn = qlist[qpos[0]]
                    fn()
                    qpos[0] += 1
                    cum[0] += c

            # ---------------- stream ----------------
            avpack = os.environ.get("MHA_AVPACK", "0") == "1"

            def emit_av(hp, av, pex, ktp, first, last):
                if avpack:
                    for head, half in ((0, 0), (1, 1), (0, 1), (1, 0)):
                        r0 = half * 64
                        nc.tensor.matmul(
                            av[head][:],
                            mm(v1[r0 : r0 + 64, ktp, 2 * hp + head, :]),
                            mm(pex[r0 : r0 + 64, head * QB : (head + 1) * QB]),
                            start=(first and half == head),
                            stop=(last and half != head),
                        )
                    return
                for head in range(2):
                    nc.tensor.matmul(
                        av[head][:],
                        mm(v1[:, ktp, 2 * hp + head, :]),
                        mm(pex[:, head * QB : (head + 1) * QB]),
                        start=first,
                        stop=last,
                    )

            def emit_tail(u, av):
                qb, hp = divmod(u, CB)
                out_sb = out_pool.tile(
                    [P, NJ, P], f32, tag="outsb", name=f"outsb{u}"
                )
                for head in range(2):
                    o_sb = o_pool.tile(
                        [HD + 1, QB], f32, tag="osb", name=f"osb{u}_{head}"
                    )
                    if NBLK > 1:
                        nc.vector.scalar_tensor_tensor(
                            o_sb[:],
                            av[head][:],
                            1.0,
                            o_acc[:, u, head, :],
                            op0=mybir.AluOpType.mult,
                            op1=mybir.AluOpType.add,
                        )
                    else:
                        nc.vector.tensor_copy(o_sb[:], av[head][:])
                    tp = ps_tr.tile([P, 4, P], f32, tag="tr", name=f"tp{u}_{head}")
                    for j in range(NJ):
                        nc.tensor.transpose(
                            tp[:, j, 0 : HD + 1],
                            o_sb[:, j * P : (j + 1) * P],
                            ident_f[: HD + 1, : HD + 1],
                        )
                    rsb = o_pool.tile([P, NJ], f32, tag="rsb", name=f"rsb{u}_{head}")
                    nc.vector.reciprocal(rsb[:], tp[:, :, HD])
                    for j in range(NJ):
                        nc.vector.tensor_scalar_mul(
                            out_sb[:, j, head * HD : (head + 1) * HD],
                            tp[:, j, 0:HD],
                            rsb[:, j : j + 1],
                        )
                q0r = qb * QB
                nc.sync.dma_start(
                    out[q0r : q0r + QB, hp * P : (hp + 1) * P].rearrange(
                        "(j p) c -> p j c", p=P
                    ),
                    out_sb[:],
                )

            n_slots = NBLK * NU * KB
            drain_slots = max(1, n_slots - NU * KB)
            pend = None  # (u, hp, av, pex, ktp, kb)
            slot = 0
            for kb in range(NBLK):
                for u in range(NU):
                    qb, hp = divmod(u, CB)
                    drain_through(("k", kb))
                    drain_through(("q", qb))
                    av = None
                    q0r = qb * QB
                    pexs = {}
                    for j in range(KB):
                        kt = kb * KB + j
                        sc_ps = ps_sc.tile([P, 2 * QB], f32, tag="sc")
                        for head in range(2):
                            r0 = head * HD
                            nc.tensor.matmul(
                                sc_ps[:, head * QB : (head + 1) * QB],
                                mm(kT[r0 : r0 + HD, hp, kt * P : (kt + 1) * P]),
                                mm(qT[r0 : r0 + HD, hp, q0r : q0r + QB]),
                                start=True,
                                stop=True,
                            )
                        pex = pex_pool.tile([P, 2 * QB], dt_x, tag="pex")
                        nc.scalar.activation(
                            pex[:], sc_ps[:], AF.Exp, bias=0.0, scale=0.125
                        )
                        pexs[j] = pex
                        if j == 0:
                            if pend is not None:
                                pu, php, pav, ppex, pktp, pkb = pend
                                drain_through(("v", pktp // KB))
                                emit_av(php, pav, ppex, pktp, False, True)
                                if pkb == NBLK - 1:
                                    # must run before this visit's av tiles
                                    # reuse the pool slots
                                    emit_tail(pu, pav)
                                elif pkb == 0 and NBLK > 1:
                                    for h in range(2):
                                        nc.vector.tensor_copy(
                                            o_acc[:, pu, h, :], pav[h][:]
                                        )
                                else:
                                    for h in range(2):
                                        nc.vector.scalar_tensor_tensor(
                                            o_acc[:, pu, h, :],
                                            pav[h][:],
                                            1.0,
                                            o_acc[:, pu, h, :],
                                            op0=mybir.AluOpType.mult,
                                            op1=mybir.AluOpType.add,
                                        )
                                pend = None
                            av = {
                                h: ps_av.tile(
                                    [HD + 1, QB],
                                    f32,
                                    tag="av",
                                    name=f"av{kb}_{u}_{h}",
                                )
                                for h in range(2)
                            }
                        else:
                            ktp = kt - 1
                            drain_through(("v", ktp // KB))
                            emit_av(hp, av, pexs[j - 1], ktp, j - 1 == 0, False)
                        budget_pop(slot, n_slots, drain_slots)
                        slot += 1
                    pend = (u, hp, av, pexs[KB - 1], kb * KB + KB - 1, kb)
            # final flush
            pu, php, pav, ppex, pktp, pkb = pend
            drain_through(("v", pktp // KB))
            emit_av(php, pav, ppex, pktp, KB == 1, True)
            emit_tail(pu, pav)
    nc.compile()
    return nc


_CACHE = {}


def _get_nc(dt_mode: str):
    key = (dt_mode, os.environ.get("MHA_V2", "0"))
    if key not in _CACHE:
        if key[1] == "1":
            _CACHE[key] = build_nc(dt_mode)
        else:
            _CACHE[key] = build_nc_v3(dt_mode)
    return _CACHE[key]


def kernel(query, key, value, Wq, bq, Wk, bk, Wv, bv, **kwargs):
    _install_ntff_hook_shim()
    from concourse.bass_utils import run_bass_kernel_spmd

    dt_mode = os.environ.get("MHA_DT", "fp16")
    nc = _get_nc(dt_mode)

    query = np.asarray(query, dtype=np.float32)
    key = np.asarray(key, dtype=np.float32)
    value = np.asarray(value, dtype=np.float32)
    Wq = np.asarray(Wq, dtype=np.float32)
    Wk = np.asarray(Wk, dtype=np.float32)
    Wv = np.asarray(Wv, dtype=np.float32)
    bq = np.asarray(bq, dtype=np.float32)
    bk = np.asarray(bk, dtype=np.float32)
    bv = np.asarray(bv, dtype=np.float32)

    in_maps = []
    for c in range(N_CORES):
        b, g = divmod(c, GROUPS)
        cs = g * C
        in_maps.append(
            {
                "xq": np.ascontiguousarray(query[b]),
                "xk": np.ascontiguousarray(key[b]),
                "xv": np.ascontiguousarray(value[b]),
                "wq": np.ascontiguousarray(Wq[:, cs : cs + C]),
                "wk": np.ascontiguousarray(Wk[:, cs : cs + C]),
                "wv": np.ascontiguousarray(Wv[:, cs : cs + C]),
                "bq": np.ascontiguousarray(bq[cs : cs + C]),
                "bk": np.ascontiguousarray(bk[cs : cs + C]),
                "bv": np.ascontiguousarray(bv[cs : cs + C]),
            }
        )

    res = run_bass_kernel_spmd(
        nc, in_maps, core_ids=list(range(N_CORES)), **kwargs
    )
    outp = np.empty((B, S, D), dtype=np.float32)
    for c in range(N_CORES):
        b, g = divmod(c, GROUPS)
        outp[b, :, g * C : (g + 1) * C] = res.results[c]["out"]
    if kwargs:
        return outp, res
    return outp



# revision 27
# speedup vs baseline: 1.0001x; 1.0001x over previous
"""Multi-head attention (B=2, S=2048, D=1024, H=16) on 8 trn2 NeuronCores.

Sharding: 2-way over batch x 4-way over head groups (4 heads / 256 cols per
core). No cross-core communication.

Per-core kernel (Tile):
  prefix:  load X_k, X_v and the first X_q block (cast f32->bf16 during DMA),
           transpose 128x128 chunks via regular matmul against identity
           (counts as PE activity so the HAM clock gate stays open), project
           kT [256, 2048] (head dim on partitions), v natural [2048, 256]
           stored as [v | 1] per ki-tile (the ones column makes the AV matmul
           also emit softmax row-sums), and qT for block 0.
  stream:  per (head-pair, qi-block of 512): for each ki-tile: S^T = kT.T@qT
           with the two heads row-packed on the PE (K=64 each) into one
           [128, 1024] psum tile (bufs=2), one exp ACTIVATE per ki-tile
           (scale=1/8 folded in), AV matmuls one ki-tile behind the scores so
           the exp stream never stalls. The remaining X_q blocks are loaded /
           transposed / projected in small slices woven into the first three
           units' ki-loops (PSUM slots shared with the AV pool). Unit
           epilogues (out^T -> out transpose + softmax normalize) are split
           in half and woven into the next unit's ki-loop the same way.
"""

import os
import sys

import numpy as np

import concourse.bass as bass
import concourse.tile as tile
from concourse import bacc, mybir
from concourse.masks import make_identity

B, S, D = 2, 2048, 1024
H, HD = 16, 64
N_CORES = 8
GROUPS = 4  # head groups (cores per batch)
NH = H // GROUPS  # local heads per core = 4
C = NH * HD  # local output cols = 256
P = 128
DB = D // P  # 8 d-chunks
CB = C // P  # 2 c-chunks (head pairs)

f32 = mybir.dt.float32
bf16 = mybir.dt.bfloat16
f32r = mybir.dt.float32r

AF = mybir.ActivationFunctionType


def _install_ntff_hook_shim():
    """Best-effort: register the axon NTFF profile hook so a traced run
    (e.g. BASS_TRACE=1) works even when the image's antenv lacks axon_hooks."""
    try:
        import antenv.axon_hooks  # noqa: F401

        return
    except ImportError:
        pass
    try:
        import types

        _hook = [None]
        mod = types.ModuleType("antenv.axon_hooks")
        mod.set_axon_ntff_profile_hook = lambda h: _hook.__setitem__(0, h)
        mod.get_axon_ntff_profile_hook = lambda: _hook[0]
        sys.modules["antenv.axon_hooks"] = mod
        from trn_agent_boot.trn_boot import _ntff_profile_via_ctypes

        so = "/opt/axon/libaxon_pjrt.so"
        if os.path.exists(so):
            mod.set_axon_ntff_profile_hook(_ntff_profile_via_ctypes(so))
    except Exception:
        pass


def build_nc(dt_mode: str = "fp16", s: int = S):
    """Trace + compile the per-core Bass kernel. dt_mode in {"fp16", "bf16", "f32r"}."""
    assert s % 512 == 0
    SB = s // P  # ki-tiles
    NBLK = s // 512  # 512-row s-blocks
    QB = 512  # qi-block
    NQB = s // QB
    NJ = QB // P  # 128-chunks per qi-block = 4
    # overlap q blocks 1.. with the attention stream only at full size
    OVERLAP_Q = SB >= 16 and NBLK == 4

    if dt_mode == "bf16":
        dt_x = bf16  # storage dtype of matmul inputs

        def mm(ap):
            return ap
    elif dt_mode == "fp16":
        dt_x = mybir.dt.float16

        def mm(ap):
            return ap
    else:
        dt_x = f32

        def mm(ap):
            return ap.bitcast(f32r)

    nc = bacc.Bacc(
        "TRN2", target_bir_lowering=False, debug=False, num_devices=N_CORES
    )

    xq = nc.dram_tensor("xq", [s, D], f32, kind="ExternalInput").ap()
    xk = nc.dram_tensor("xk", [s, D], f32, kind="ExternalInput").ap()
    xv = nc.dram_tensor("xv", [s, D], f32, kind="ExternalInput").ap()
    wq = nc.dram_tensor("wq", [D, C], f32, kind="ExternalInput").ap()
    wk = nc.dram_tensor("wk", [D, C], f32, kind="ExternalInput").ap()
    wv = nc.dram_tensor("wv", [D, C], f32, kind="ExternalInput").ap()
    bq = nc.dram_tensor("bq", [C], f32, kind="ExternalInput").ap()
    bk = nc.dram_tensor("bk", [C], f32, kind="ExternalInput").ap()
    bv = nc.dram_tensor("bv", [C], f32, kind="ExternalInput").ap()
    out = nc.dram_tensor("out", [s, C], f32, kind="ExternalOutput").ap()

    with tile.TileContext(nc) as tc:
        with (
            tc.tile_pool(name="const", bufs=1) as const_pool,
            tc.tile_pool(name="wts", bufs=1) as wts_pool,
            tc.tile_pool(name="qkv", bufs=1) as qkv_pool,
            tc.tile_pool(name="xn", bufs=4) as xn_pool,
            tc.tile_pool(name="xt", bufs=3) as xt_pool,
        ):
            ident = const_pool.tile([P, P], dt_x)
            make_identity(nc, ident[:])
            ident_f = const_pool.tile([P, P], f32)
            make_identity(nc, ident_f[:])

            # First x block's DMA goes on the queue before the weights so the
            # PE can start transposing ~6us in; weights follow (k first).
            xn_first = xn_pool.tile([P, 4, D], dt_x, tag="xn", name="xn_first")
            for t in range(4):
                nc.gpsimd.dma_start(
                    xn_first[:, t, :],
                    xk[t * P : (t + 1) * P, :].rearrange("(t p) d -> p t d", p=P)[
                        :, 0
                    ],
                )

            # weights: [p, dc, c] where d = dc*128 + p
            w_sb = {}
            for name, ap in (("k", wk), ("v", wv), ("q", wq)):
                t = wts_pool.tile([P, DB, C], dt_x, tag=f"w_{name}", name=f"w_{name}")
                nc.gpsimd.dma_start(t[:], ap.rearrange("(dc p) c -> p dc c", p=P))
                w_sb[name] = t
            # biases for q/k: [p, cc] with c = cc*128 + p
            b_sb = {}
            for name, ap in (("q", bq), ("k", bk)):
                t = const_pool.tile([P, CB], f32, tag=f"b_{name}", name=f"b_{name}")
                nc.sync.dma_start(t[:], ap.rearrange("(cc p) -> p cc", p=P))
                b_sb[name] = t
            # v bias as a row vector + ones row for the K=1 bias matmul
            bv_row = const_pool.tile([1, C], dt_x)
            nc.gpsimd.dma_start(bv_row[:], bv[None, :])
            ones_row = const_pool.tile([1, P], dt_x)
            nc.vector.memset(ones_row[:], 1.0)

            # projection outputs (persistent)
            qT = qkv_pool.tile([P, CB, s], dt_x)  # q^T: [c%128, c//128, s]
            kT = qkv_pool.tile([P, CB, s], dt_x)
            v1 = qkv_pool.tile([P, SB, NH, HD + 1], dt_x)  # [ki%128, ki//128, h, d|1]
            nc.vector.memset(v1[:, :, :, HD : HD + 1], 1.0)

            def emit_xn_dma(x_ap, blk):
                xn = xn_pool.tile([P, 4, D], dt_x, tag="xn")
                src = x_ap[blk * 512 : (blk + 1) * 512, :].rearrange(
                    "(t p) d -> p t d", p=P
                )
                for t in range(4):
                    nc.gpsimd.dma_start(xn[:, t, :], src[:, t, :])
                return xn

            def emit_qk_proj_cc(name, blk, xt, cc, pj_tile):
                dsttile = qT if name == "q" else kT
                for dc in range(DB):
                    nc.tensor.matmul(
                        pj_tile[:],
                        mm(w_sb[name][:, dc, cc * P : (cc + 1) * P]),
                        mm(xt[:, dc, :]),
                        start=(dc == 0),
                        stop=(dc == DB - 1),
                    )
                nc.vector.tensor_scalar_add(
                    dsttile[:, cc, blk * 512 : (blk + 1) * 512],
                    pj_tile[:],
                    b_sb[name][:, cc : cc + 1],
                )

            # ---------------- prefix: k, v, q-block-0 ----------------
            with (
                tc.tile_pool(name="ps_tr", bufs=2, space="PSUM") as ps_tr,
                tc.tile_pool(name="ps_pj", bufs=2, space="PSUM") as ps_pj,
                tc.tile_pool(name="ps_pv", bufs=2, space="PSUM") as ps_pv,
            ):
                n_evict = 0

                def emit_proj(name, blk, xt):
                    if name in ("q", "k"):
                        for cc in range(CB):
                            ps = ps_pj.tile([P, 512], f32, tag="pj")
                            emit_qk_proj_cc(name, blk, xt, cc, ps)
                    else:
                        for t in range(4):
                            sc = blk * 4 + t
                            ps = ps_pv.tile([P, C], f32, tag="pv")
                            for dc in range(DB):
                                nc.tensor.matmul(
                                    ps[:],
                                    mm(xt[:, dc, t * P : (t + 1) * P]),
                                    mm(w_sb["v"][:, dc, :]),
                                    start=(dc == 0),
                                    stop=False,
                                )
                            nc.tensor.matmul(
                                ps[:],
                                mm(ones_row[:, :]),
                                mm(bv_row[:, :]),
                                start=False,
                                stop=True,
                            )
                            nc.vector.tensor_copy(
                                v1[:, sc, :, 0:HD],
                                ps.rearrange("p (h e) -> p h e", h=NH),
                            )

                prefix_items = [("k", xk, blk) for blk in range(NBLK)]
                prefix_items += [("v", xv, blk) for blk in range(NBLK)]
                prefix_items += [
                    ("q", xq, blk) for blk in range(1 if OVERLAP_Q else NBLK)
                ]
                pending = None  # (name, blk, xt) with projections still to emit
                for name, x_ap, blk in prefix_items:
                    if name == "k" and blk == 0:
                        xn = xn_first
                    else:
                        xn = emit_xn_dma(x_ap, blk)
                    xt = xt_pool.tile([P, DB, 512], dt_x, tag="xt")
                    for t in range(4):
                        # 8 transposed chunks into one [128, 8, 128] psum
                        # tile, evicted with a single wide copy.
                        ps = ps_tr.tile([P, DB, P], f32, tag="tr")
                        for dc in range(DB):
                            nc.tensor.matmul(
                                ps[:, dc, :],
                                mm(xn[:, t, dc * P : (dc + 1) * P]),
                                mm(ident[:]),
                                start=True,
                                stop=True,
                            )
                        dst = xt.rearrange("p dc (t q) -> p t dc q", q=P)[:, t]
                        nc.vector.tensor_copy(dst, ps[:])
                        n_evict += 1
                    if pending is not None:
                        emit_proj(*pending)
                    pending = (name, blk, xt)
                emit_proj(*pending)

            # ---------------- attention stream ----------------
            with (
                tc.tile_pool(name="ps_sc", bufs=2, space="PSUM") as ps_sc,
                tc.tile_pool(name="ps_av", bufs=4, space="PSUM") as ps_av,
                tc.tile_pool(name="pexp", bufs=3) as p_pool,
                tc.tile_pool(name="osb", bufs=2) as o_pool,
                tc.tile_pool(name="outsb", bufs=2) as out_pool,
            ):

                avpack = os.environ.get("MHA_AVPACK", "1") == "1"

                def emit_av(hp, av, pex, ktp, last):
                    if not avpack:
                        for head in range(2):
                            nc.tensor.matmul(
                                av[head][:],
                                mm(v1[:, ktp, 2 * hp + head, :]),
                                mm(pex[:, head * QB : (head + 1) * QB]),
                                start=(ktp == 0),
                                stop=last,
                            )
                        return
                    # The K=128 AV contraction is split into two K=64 halves on
                    # distinct PE row groups (tile_position (0,0)/(64,0), auto-
                    # inferred from base partitions). Pairs (h0,half0)+(h1,half1)
                    # and (h0,half1)+(h1,half0) run concurrently on the array,
                    # halving the AV stream cost.
                    order = (
                        ((0, 0), (1, 1), (0, 1), (1, 0))
                        if os.environ.get("MHA_AVORD", "0") == "0"
                        else ((0, 0), (0, 1), (1, 1), (1, 0))
                    )
                    first = {}
                    lastmm = {}
                    for head, half in order:
                        first.setdefault(head, (head, half))
                        lastmm[head] = (head, half)
                    for head, half in order:
                        r0 = half * 64
                        nc.tensor.matmul(
                            av[head][:],
                            mm(v1[r0 : r0 + 64, ktp, 2 * hp + head, :]),
                            mm(pex[r0 : r0 + 64, head * QB : (head + 1) * QB]),
                            start=(ktp == 0 and first[head] == (head, half)),
                            stop=(last and lastmm[head] == (head, half)),
                        )

                def emit_tail_half(hp, qb, av, head, out_sb):
                    o_sb = o_pool.tile(
                        [HD + 1, QB], f32, tag="osb", name=f"osb{hp}_{qb}_{head}"
                    )
                    nc.vector.tensor_copy(o_sb[:], av[head][:])
                    tp = ps_av.tile(
                        [P, NJ, HD + 1],
                        f32,
                        tag="av",
                        name=f"tp{hp}_{qb}_{head}",
                    )
                    for j in range(NJ):
                        nc.tensor.transpose(
                            tp[:, j, :],
                            o_sb[:, j * P : (j + 1) * P],
                            ident_f[: HD + 1, : HD + 1],
                        )
                    rsb = o_pool.tile(
                        [P, NJ], f32, tag="rsb", name=f"rsb{hp}_{qb}_{head}"
                    )
                    nc.vector.reciprocal(rsb[:], tp[:, :, HD])
                    for j in range(NJ):
                        nc.vector.tensor_scalar_mul(
                            out_sb[:, j, head * HD : (head + 1) * HD],
                            tp[:, j, 0:HD],
                            rsb[:, j : j + 1],
                        )

                def emit_tail_dma(hp, qb, out_sb):
                    q0 = qb * QB
                    nc.sync.dma_start(
                        out[q0 : q0 + QB, hp * P : (hp + 1) * P].rearrange(
                            "(j p) c -> p j c", p=P
                        ),
                        out_sb[:],
                    )

                # woven q-block work: unit index -> q block to process
                qwork = {}
                if OVERLAP_Q:
                    for u, blk in enumerate(range(1, NBLK)):
                        qwork[u] = blk
                qstate = {}  # per live q block: dict(xn=, xt=, pj=)

                def emit_qwork(blk, kt):
                    st = qstate[blk]
                    if kt == 0:
                        st["xn"] = emit_xn_dma(xq, blk)
                        st["xt"] = xt_pool.tile(
                            [P, DB, 512], dt_x, tag="xt", name=f"xt_q{blk}"
                        )
                    elif 3 <= kt <= 6:
                        t = kt - 3
                        for dhalf in range(2):
                            tr = ps_av.tile(
                                [P, 4, P],
                                f32,
                                tag="av",
                                name=f"tr_q{blk}_{t}_{dhalf}",
                            )
                            for i in range(4):
                                dc = dhalf * 4 + i
                                nc.tensor.matmul(
                                    tr[:, i, :],
                                    mm(st["xn"][:, t, dc * P : (dc + 1) * P]),
                                    mm(ident[:]),
                                    start=True,
                                    stop=True,
                                )
                            nc.vector.tensor_copy(
                                st["xt"][
                                    :, dhalf * 4 : dhalf * 4 + 4, t * P : (t + 1) * P
                                ],
                                tr[:],
                            )
                    elif 7 <= kt <= 14:
                        cc, half = divmod(kt - 7, 4)
                        if half == 0:
                            st["pj"] = ps_av.tile(
                                [P, 512], f32, tag="av", name=f"pj_q{blk}_{cc}"
                            )
                        for dc in range(half * 2, half * 2 + 2):
                            nc.tensor.matmul(
                                st["pj"][:],
                                mm(w_sb["q"][:, dc, cc * P : (cc + 1) * P]),
                                mm(st["xt"][:, dc, :]),
                                start=(dc == 0),
                                stop=(dc == DB - 1),
                            )
                        if half == 3:
                            nc.vector.tensor_scalar_add(
                                qT[:, cc, blk * 512 : (blk + 1) * 512],
                                st["pj"][:],
                                b_sb["q"][:, cc : cc + 1],
                            )
                            del st["pj"]

                KT_A = max(1, SB // 8)
                KT_B = max(KT_A + 1, min(4, SB - 1))
                tail_prev = None  # (hp, qb, av) of the finished unit
                tail_outsb = None
                uidx = 0
                for hp in range(CB):  # head pair (c-chunk)
                    for qb in range(NQB):  # qi block of 512
                        q0 = qb * QB
                        if uidx in qwork:
                            qstate[qwork[uidx]] = {}
                        av = {}
                        for head in range(2):
                            av[head] = ps_av.tile(
                                [HD + 1, QB], f32, tag="av", name=f"av{hp}_{qb}_{head}"
                            )
                        # scores/exp stream one ki-tile ahead of the AV
                        # matmuls so the ACT exp stream never stalls on PE.
                        pex_q = []
                        for kt in range(SB):
                            sc_ps = ps_sc.tile([P, 2 * QB], f32, tag="sc")
                            for head in range(2):
                                r0 = head * HD
                                nc.tensor.matmul(
                                    sc_ps[:, head * QB : (head + 1) * QB],
                                    mm(kT[r0 : r0 + HD, hp, kt * P : (kt + 1) * P]),
                                    mm(qT[r0 : r0 + HD, hp, q0 : q0 + QB]),
                                    start=True,
                                    stop=True,
                                )
                            pex = p_pool.tile([P, 2 * QB], dt_x, tag="pex")
                            nc.scalar.activation(
                                pex[:], sc_ps[:], AF.Exp, bias=0.0, scale=0.125
                            )
                            pex_q.append(pex)
                            if kt >= 1:
                                emit_av(hp, av, pex_q[kt - 1], kt - 1, False)
                            if kt == KT_A and tail_prev is not None:
                                tail_outsb = out_pool.tile(
                                    [P, NJ, P],
                                    f32,
                                    tag="outsb",
                                    name=f"outsb{tail_prev[0]}_{tail_prev[1]}",
                                )
                                emit_tail_half(*tail_prev, 0, tail_outsb)
                            if kt == KT_B and tail_prev is not None:
                                emit_tail_half(*tail_prev, 1, tail_outsb)
                                emit_tail_dma(tail_prev[0], tail_prev[1], tail_outsb)
                                tail_prev = None
                            if uidx in qwork:
                                emit_qwork(qwork[uidx], kt)
                        emit_av(hp, av, pex_q[SB - 1], SB - 1, True)
                        tail_prev = (hp, qb, av)
                        uidx += 1
                tail_outsb = out_pool.tile(
                    [P, NJ, P], f32, tag="outsb", name="outsb_last"
                )
                emit_tail_half(*tail_prev, 0, tail_outsb)
                emit_tail_half(*tail_prev, 1, tail_outsb)
                emit_tail_dma(tail_prev[0], tail_prev[1], tail_outsb)
    nc.compile()
    return nc


def build_nc_v3(dt_mode: str = "fp16", s: int = S):
    """Sweep-structured kernel: kt-block-outer so the softmax exp stream (the
    ScalarE wall, ~147us) starts ~16us in and never starves.

    Stream = NBLK sweeps x NU units x KB kt-tiles. AV partials accumulate in
    PSUM within a sweep-visit and are folded into an SBUF accumulator between
    sweeps. All input-block production (DMA, PE transposes, projections) except
    (k0, q0) is woven into the stream's PE slack via a deadline-forced work
    queue. PSUM: 4 banks scores (double-buffered) + 2 AV + 1 transpose + 1
    projection.
    """
    assert s % 512 == 0
    SB = s // P
    NBLK = s // 512
    KB = SB // NBLK  # 4 kt per sweep visit
    QB = 512
    NQB = s // QB
    NU = NQB * CB  # units: u -> (qb, hp)
    NJ = QB // P

    if dt_mode == "bf16":
        dt_x = bf16

        def mm(ap):
            return ap
    elif dt_mode == "fp16":
        dt_x = mybir.dt.float16

        def mm(ap):
            return ap
    else:
        dt_x = f32

        def mm(ap):
            return ap.bitcast(f32r)

    nc = bacc.Bacc(
        "TRN2", target_bir_lowering=False, debug=False, num_devices=N_CORES
    )

    xq = nc.dram_tensor("xq", [s, D], f32, kind="ExternalInput").ap()
    xk = nc.dram_tensor("xk", [s, D], f32, kind="ExternalInput").ap()
    xv = nc.dram_tensor("xv", [s, D], f32, kind="ExternalInput").ap()
    wq = nc.dram_tensor("wq", [D, C], f32, kind="ExternalInput").ap()
    wk = nc.dram_tensor("wk", [D, C], f32, kind="ExternalInput").ap()
    wv = nc.dram_tensor("wv", [D, C], f32, kind="ExternalInput").ap()
    bq = nc.dram_tensor("bq", [C], f32, kind="ExternalInput").ap()
    bk = nc.dram_tensor("bk", [C], f32, kind="ExternalInput").ap()
    bv = nc.dram_tensor("bv", [C], f32, kind="ExternalInput").ap()
    out = nc.dram_tensor("out", [s, C], f32, kind="ExternalOutput").ap()
    x_aps = {"q": xq, "k": xk, "v": xv}
    w_aps = {"q": wq, "k": wk, "v": wv}

    with tile.TileContext(nc) as tc:
        with (
            tc.tile_pool(name="const", bufs=1) as const_pool,
            tc.tile_pool(name="wts", bufs=1) as wts_pool,
            tc.tile_pool(name="qkv", bufs=1) as qkv_pool,
            tc.tile_pool(name="xn", bufs=4) as xn_pool,
            tc.tile_pool(name="xt", bufs=2) as xt_pool,
            tc.tile_pool(name="pex", bufs=6) as pex_pool,
            tc.tile_pool(name="osb", bufs=2) as o_pool,
            tc.tile_pool(name="outsb", bufs=2) as out_pool,
            tc.tile_pool(name="ps_sc", bufs=2, space="PSUM") as ps_sc,
            tc.tile_pool(name="ps_av", bufs=2, space="PSUM") as ps_av,
            tc.tile_pool(name="ps_wk", bufs=2, space="PSUM") as ps_wk,
        ):
            ident = const_pool.tile([P, P], dt_x)
            make_identity(nc, ident[:])
            ident_f = const_pool.tile([P, P], f32)
            make_identity(nc, ident_f[:])
            ones_row = const_pool.tile([1, P], dt_x)
            nc.vector.memset(ones_row[:], 1.0)
            # warm the exp table set during the DMA-bound prefix
            warm = const_pool.tile([1, 8], f32)
            nc.vector.memset(warm[:], 0.0)
            warm2 = const_pool.tile([1, 8], dt_x)
            nc.scalar.activation(warm2[:], warm[:], AF.Exp, bias=0.0, scale=1.0)

            qT = qkv_pool.tile([P, CB, s], dt_x)
            kT = qkv_pool.tile([P, CB, s], dt_x)
            v1 = qkv_pool.tile([P, SB, NH, HD + 1], dt_x)
            nc.vector.memset(v1[:, :, :, HD : HD + 1], 1.0)
            o_acc = None
            if NBLK > 1:
                o_acc = qkv_pool.tile([HD + 1, NU, 2, QB], f32, name="o_acc")

            w_sb = {}
            b_sb = {}
            bv_row = const_pool.tile([1, C], dt_x)
            bstate = {}

            def emit_dma(name, blk):
                xn = xn_pool.tile([P, 4, D], dt_x, tag="xn", name=f"xn_{name}{blk}")
                src = x_aps[name][blk * 512 : (blk + 1) * 512, :].rearrange(
                    "(t p) d -> p t d", p=P
                )
                for t in range(4):
                    nc.gpsimd.dma_start(xn[:, t, :], src[:, t, :])
                bstate[(name, blk)]["xn"] = xn

            def emit_w(name):
                t = wts_pool.tile([P, DB, C], dt_x, tag=f"w_{name}", name=f"w_{name}")
                nc.gpsimd.dma_start(
                    t[:], w_aps[name].rearrange("(dc p) c -> p dc c", p=P)
                )
                w_sb[name] = t

            def emit_tr(name, blk, t, half):
                st = bstate[(name, blk)]
                if "xt" not in st:
                    st["xt"] = xt_pool.tile(
                        [P, DB, 512], dt_x, tag="xt", name=f"xt_{name}{blk}"
                    )
                ps = ps_wk.tile([P, 512], f32, tag="wk")
                psv = ps.rearrange("p (i q) -> p i q", q=P)
                for i in range(4):
                    dc = half * 4 + i
                    nc.tensor.matmul(
                        psv[:, i, :],
                        mm(st["xn"][:, t, dc * P : (dc + 1) * P]),
                        mm(ident[:]),
                        start=True,
                        stop=True,
                    )
                nc.vector.tensor_copy(
                    st["xt"][:, half * 4 : half * 4 + 4, t * P : (t + 1) * P],
                    psv[:],
                )

            def emit_pj(name, blk, cc, half):
                st = bstate[(name, blk)]
                dsttile = qT if name == "q" else kT
                if half == 0:
                    st[f"pj{cc}"] = ps_wk.tile(
                        [P, 512], f32, tag="wk", name=f"pj_{name}{blk}_{cc}"
                    )
                ps = st[f"pj{cc}"]
                for dc in range(half * 4, half * 4 + 4):
                    nc.tensor.matmul(
                        ps[:],
                        mm(w_sb[name][:, dc, cc * P : (cc + 1) * P]),
                        mm(st["xt"][:, dc, :]),
                        start=(dc == 0),
                        stop=(dc == DB - 1),
                    )
                if half == 1:
                    nc.vector.tensor_scalar_add(
                        dsttile[:, cc, blk * 512 : (blk + 1) * 512],
                        ps[:],
                        b_sb[name][:, cc : cc + 1],
                    )
                    del st[f"pj{cc}"]

            def emit_pv(blk, t):
                st = bstate[("v", blk)]
                sc = blk * 4 + t
                ps = ps_wk.tile([P, 512], f32, tag="wk")
                for dc in range(DB):
                    nc.tensor.matmul(
                        ps[:, 0:C],
                        mm(st["xt"][:, dc, t * P : (t + 1) * P]),
                        mm(w_sb["v"][:, dc, :]),
                        start=(dc == 0),
                        stop=False,
                    )
                nc.tensor.matmul(
                    ps[:, 0:C],
                    mm(ones_row[:, :]),
                    mm(bv_row[:, :]),
                    start=False,
                    stop=True,
                )
                nc.vector.tensor_copy(
                    v1[:, sc, :, 0:HD],
                    ps[:, 0:C].rearrange("p (h e) -> p h e", h=NH),
                )

            def block_items(name, blk):
                items = []
                for t in range(4):
                    for half in range(2):
                        items.append(
                            (0.45, (lambda n, b, tt, hh: lambda: emit_tr(n, b, tt, hh))(name, blk, t, half))
                        )
                if name in ("q", "k"):
                    for cc in range(CB):
                        for half in range(2):
                            items.append(
                                (0.9, (lambda n, b, c, hh: lambda: emit_pj(n, b, c, hh))(name, blk, cc, half))
                            )
                else:
                    for t in range(4):
                        items.append(
                            (1.0, (lambda b, tt: lambda: emit_pv(b, tt))(blk, t))
                        )
                return items

            # ---------------- prefix ----------------
            for (name, blk) in [(n, b) for n in ("q", "k", "v") for b in range(NBLK)]:
                bstate[(name, blk)] = {}
            emit_dma("k", 0)
            emit_w("k")
            emit_dma("q", 0)
            emit_w("q")
            emit_dma("v", 0)
            emit_w("v")
            nc.gpsimd.dma_start(bv_row[:], bv[None, :])
            for name, ap in (("q", bq), ("k", bk)):
                t = const_pool.tile([P, CB], f32, tag=f"b_{name}", name=f"b_{name}")
                nc.sync.dma_start(t[:], ap.rearrange("(cc p) -> p cc", p=P))
                b_sb[name] = t
            for cost, fn in block_items("k", 0) + block_items("q", 0):
                fn()

            # ---------------- weave queue ----------------
            queue_blocks = [("v", 0)]
            queue_blocks += [("q", b) for b in range(1, NQB)]
            for b in range(1, NBLK):
                queue_blocks += [("k", b), ("v", b)]
            qitems = {key: block_items(*key) for key in queue_blocks}
            # DMA for block i leads by one queue position
            for i, key in enumerate(queue_blocks):
                lead = queue_blocks[max(0, i - 1)]
                dma_fn = (lambda k: lambda: emit_dma(*k))(key)
                qitems[lead].insert(0, (0.15, dma_fn))
            qlist = [(key, cost, fn) for key in queue_blocks for cost, fn in qitems[key]]
            qpos = [0]  # next index into qlist
            total_cost = sum(c for _, c, _ in qlist)
            done_upto = {}
            for i, (key, _, _) in enumerate(qlist):
                done_upto[key] = i + 1  # drain-through index per block

            def drain_through(key):
                tgt = done_upto.get(key, 0)
                while qpos[0] < tgt:
                    _, _, fn = qlist[qpos[0]]
                    fn()
                    qpos[0] += 1

            cum = [0.0]
            # deadline-aware drain plan: piecewise-linear cum-cost targets so
            # forced drains never burst (bursts block scores in the PE FIFO
            # and starve the exp stream)
            block_deadline = {}
            for key in queue_blocks:
                name, b = key
                if name == "v":
                    block_deadline[key] = max(2, b * NU * KB - 1)
                elif name == "q":
                    block_deadline[key] = max(2, b * CB * KB - 1)
                else:  # k
                    block_deadline[key] = max(2, b * NU * KB - 3)
            plan = []  # (slot, cum_cost_required)
            run = 0.0
            for key in queue_blocks:
                run += sum(c for c, _ in qitems[key])
                plan.append((block_deadline[key], run))
            plan.sort()

            def plan_target(slot):
                prev_s, prev_c = 0, 0.0
                for ds, dc in plan:
                    if slot < ds:
                        return prev_c + (dc - prev_c) * (slot + 1 - prev_s) / max(
                            1, ds - prev_s
                        )
                    prev_s, prev_c = ds, dc
                return total_cost

            def budget_pop(slot, n_slots, drain_slots):
                tgt = max(
                    plan_target(slot),
                    total_cost * min(1.0, (slot + 1) / max(1, drain_slots)),
                )
                while qpos[0] < len(qlist) and cum[0] < tgt:
                    _, c, fn = qlist[qpos[0]]
                    fn()
                    qpos[0] += 1
                    cum[0] += c

            # ---------------- stream ----------------
            avpack = os.environ.get("MHA_AVPACK", "0") == "1"

            def emit_av(hp, av, pex, ktp, first, last):
                if avpack:
                    for head, half in ((0, 0), (1, 1), (0, 1), (1, 0)):
                        r0 = half * 64
                        nc.tensor.matmul(
                            av[head][0 : HD + 1, :],
                            mm(v1[r0 : r0 + 64, ktp, 2 * hp + head, :]),
                            mm(pex[r0 : r0 + 64, head * QB : (head + 1) * QB]),
                            start=(first and half == head),
                            stop=(last and half != head),
                        )
                    return
                for head in range(2):
                    nc.tensor.matmul(
                        av[head][0 : HD + 1, :],
                        mm(v1[:, ktp, 2 * hp + head, :]),
                        mm(pex[:, head * QB : (head + 1) * QB]),
                        start=first,
                        stop=last,
                    )

            def emit_tail(u, av):
                qb, hp = divmod(u, CB)
                out_sb = out_pool.tile(
                    [P, NJ, P], f32, tag="outsb", name=f"outsb{u}"
                )
                for head in range(2):
                    o_sb = o_pool.tile(
                        [HD + 1, QB], f32, tag="osb", name=f"osb{u}_{head}"
                    )
                    if NBLK > 1:
                        nc.vector.scalar_tensor_tensor(
                            o_sb[:],
                            av[head][0 : HD + 1, :],
                            1.0,
                            o_acc[:, u, head, :],
                            op0=mybir.AluOpType.mult,
                            op1=mybir.AluOpType.add,
                        )
                    else:
                        nc.vector.tensor_copy(o_sb[:], av[head][0 : HD + 1, :])
                    tp = ps_av.tile([P, 512], f32, tag="av", name=f"tp{u}_{head}")
                    tpv = tp.rearrange("p (i q) -> p i q", q=P)
                    for j in range(NJ):
                        nc.tensor.transpose(
                            tpv[:, j, 0 : HD + 1],
                            o_sb[:, j * P : (j + 1) * P],
                            ident_f[: HD + 1, : HD + 1],
                        )
                    rsb = o_pool.tile([P, NJ], f32, tag="rsb", name=f"rsb{u}_{head}")
                    nc.vector.reciprocal(rsb[:], tpv[:, :, HD])
                    for j in range(NJ):
                        nc.vector.tensor_scalar_mul(
                            out_sb[:, j, head * HD : (head + 1) * HD],
                            tpv[:, j, 0:HD],
                            rsb[:, j : j + 1],
                        )
                q0r = qb * QB
                nc.sync.dma_start(
                    out[q0r : q0r + QB, hp * P : (hp + 1) * P].rearrange(
                        "(j p) c -> p j c", p=P
                    ),
                    out_sb[:],
                )

            n_slots = NBLK * NU * KB
            drain_slots = max(1, n_slots - NU * KB)
            pend = None  # (u, hp, av, pex, ktp, kb)
            slot = 0
            for kb in range(NBLK):
                for u in range(NU):
                    qb, hp = divmod(u, CB)
                    drain_through(("k", kb))
                    drain_through(("q", qb))
                    av = None
                    q0r = qb * QB
                    pexs = {}
                    for j in range(KB):
                        kt = kb * KB + j
                        sc_ps = ps_sc.tile([P, 2 * QB], f32, tag="sc")
                        for head in range(2):
                            r0 = head * HD
                            nc.tensor.matmul(
                                sc_ps[:, head * QB : (head + 1) * QB],
                                mm(kT[r0 : r0 + HD, hp, kt * P : (kt + 1) * P]),
                                mm(qT[r0 : r0 + HD, hp, q0r : q0r + QB]),
                                start=True,
                                stop=True,
                            )
                        pex = pex_pool.tile([P, 2 * QB], dt_x, tag="pex")
                        nc.scalar.activation(
                            pex[:], sc_ps[:], AF.Exp, bias=0.0, scale=0.125
                        )
                        pexs[j] = pex
                        if j == 0:
                            if pend is not None:
                                pu, php, pav, ppex, pktp, pkb = pend
                                drain_through(("v", pktp // KB))
                                emit_av(php, pav, ppex, pktp, False, True)
                                if pkb == NBLK - 1:
                                    # must run before this visit's av tiles
                                    # reuse the pool slots
                                    emit_tail(pu, pav)
                                elif pkb == 0 and NBLK > 1:
                                    for h in range(2):
                                        nc.vector.tensor_copy(
                                            o_acc[:, pu, h, :],
                                            pav[h][0 : HD + 1, :],
                                        )
                                else:
                                    for h in range(2):
                                        nc.vector.scalar_tensor_tensor(
                                            o_acc[:, pu, h, :],
                                            pav[h][0 : HD + 1, :],
                                            1.0,
                                            o_acc[:, pu, h, :],
                                            op0=mybir.AluOpType.mult,
                                            op1=mybir.AluOpType.add,
                                        )
                                pend = None
                            av = {
                                h: ps_av.tile(
                                    [P, 512],
                                    f32,
                                    tag="av",
                                    name=f"av{kb}_{u}_{h}",
                                )
                                for h in range(2)
                            }
                        else:
                            ktp = kt - 1
                            drain_through(("v", ktp // KB))
                            emit_av(hp, av, pexs[j - 1], ktp, j - 1 == 0, False)
                        budget_pop(slot, n_slots, drain_slots)
                        slot += 1
                    pend = (u, hp, av, pexs[KB - 1], kb * KB + KB - 1, kb)
            # final flush
            pu, php, pav, ppex, pktp, pkb = pend
            drain_through(("v", pktp // KB))
            emit_av(php, pav, ppex, pktp, KB == 1, True)
            emit_tail(pu, pav)
    nc.compile()
    return nc


_CACHE = {}


def _get_nc(dt_mode: str):
    key = (dt_mode, os.environ.get("MHA_V2", "0"))
    if key not in _CACHE:
        if key[1] == "1":
            _CACHE[key] = build_nc(dt_mode)
        else:
            _CACHE[key] = build_nc_v3(dt_mode)
    return _CACHE[key]


def kernel(query, key, value, Wq, bq, Wk, bk, Wv, bv, **kwargs):
    _install_ntff_hook_shim()
    from concourse.bass_utils import run_bass_kernel_spmd

    dt_mode = os.environ.get("MHA_DT", "fp16")
    nc = _get_nc(dt_mode)

    query = np.asarray(query, dtype=np.float32)
    key = np.asarray(key, dtype=np.float32)
    value = np.asarray(value, dtype=np.float32)
    Wq = np.asarray(Wq, dtype=np.float32)
    Wk = np.asarray(Wk, dtype=np.float32)
    Wv = np.asarray(Wv, dtype=np.float32)
    bq = np.asarray(bq, dtype=np.float32)
    bk = np.asarray(bk, dtype=np.float32)
    bv = np.asarray(bv, dtype=np.float32)

    in_maps = []
    for c in range(N_CORES):
        b, g = divmod(c, GROUPS)
        cs = g * C
        in_maps.append(
            {
                "xq": np.ascontiguousarray(query[b]),
                "xk": np.ascontiguousarray(key[b]),
                "xv": np.ascontiguousarray(value[b]),
                "wq": np.ascontiguousarray(Wq[:, cs : cs + C]),
                "wk": np.ascontiguousarray(Wk[:, cs : cs + C]),
                "wv": np.ascontiguousarray(Wv[:, cs : cs + C]),
                "bq": np.ascontiguousarray(bq[cs : cs + C]),
                "bk": np.ascontiguousarray(bk[cs : cs + C]),
                "bv": np.ascontiguousarray(bv[cs : cs + C]),
            }
        )

    res = run_bass_kernel_spmd(
        nc, in_maps, core_ids=list(range(N_CORES)), **kwargs
    )
    outp = np.empty((B, S, D), dtype=np.float32)
    for c in range(N_CORES):
        b, g = divmod(c, GROUPS)
        outp[b, :, g * C : (g + 1) * C] = res.results[c]["out"]
    if kwargs:
        return outp, res
    return outp



# revision 35
# speedup vs baseline: 1.0044x; 1.0043x over previous
"""Multi-head attention (B=2, S=2048, D=1024, H=16) on 8 trn2 NeuronCores.

Sharding: 2-way over batch x 4-way over head groups (4 heads / 256 cols per
core). No cross-core communication.

Per-core kernel (Tile):
  prefix:  load X_k, X_v and the first X_q block (cast f32->bf16 during DMA),
           transpose 128x128 chunks via regular matmul against identity
           (counts as PE activity so the HAM clock gate stays open), project
           kT [256, 2048] (head dim on partitions), v natural [2048, 256]
           stored as [v | 1] per ki-tile (the ones column makes the AV matmul
           also emit softmax row-sums), and qT for block 0.
  stream:  per (head-pair, qi-block of 512): for each ki-tile: S^T = kT.T@qT
           with the two heads row-packed on the PE (K=64 each) into one
           [128, 1024] psum tile (bufs=2), one exp ACTIVATE per ki-tile
           (scale=1/8 folded in), AV matmuls one ki-tile behind the scores so
           the exp stream never stalls. The remaining X_q blocks are loaded /
           transposed / projected in small slices woven into the first three
           units' ki-loops (PSUM slots shared with the AV pool). Unit
           epilogues (out^T -> out transpose + softmax normalize) are split
           in half and woven into the next unit's ki-loop the same way.
"""

import os
import sys

import numpy as np

import concourse.bass as bass
import concourse.tile as tile
from concourse import bacc, mybir
from concourse.masks import make_identity

B, S, D = 2, 2048, 1024
H, HD = 16, 64
N_CORES = 8
GROUPS = 4  # head groups (cores per batch)
NH = H // GROUPS  # local heads per core = 4
C = NH * HD  # local output cols = 256
P = 128
DB = D // P  # 8 d-chunks
CB = C // P  # 2 c-chunks (head pairs)

f32 = mybir.dt.float32
bf16 = mybir.dt.bfloat16
f32r = mybir.dt.float32r

AF = mybir.ActivationFunctionType


def _install_ntff_hook_shim():
    """Best-effort: register the axon NTFF profile hook so a traced run
    (e.g. BASS_TRACE=1) works even when the image's antenv lacks axon_hooks."""
    try:
        import antenv.axon_hooks  # noqa: F401

        return
    except ImportError:
        pass
    try:
        import types

        _hook = [None]
        mod = types.ModuleType("antenv.axon_hooks")
        mod.set_axon_ntff_profile_hook = lambda h: _hook.__setitem__(0, h)
        mod.get_axon_ntff_profile_hook = lambda: _hook[0]
        sys.modules["antenv.axon_hooks"] = mod
        from trn_agent_boot.trn_boot import _ntff_profile_via_ctypes

        so = "/opt/axon/libaxon_pjrt.so"
        if os.path.exists(so):
            mod.set_axon_ntff_profile_hook(_ntff_profile_via_ctypes(so))
    except Exception:
        pass


def build_nc(dt_mode: str = "fp16", s: int = S):
    """Trace + compile the per-core Bass kernel. dt_mode in {"fp16", "bf16", "f32r"}."""
    assert s % 512 == 0
    SB = s // P  # ki-tiles
    NBLK = s // 512  # 512-row s-blocks
    QB = 512  # qi-block
    NQB = s // QB
    NJ = QB // P  # 128-chunks per qi-block = 4
    # overlap q blocks 1.. with the attention stream only at full size
    OVERLAP_Q = SB >= 16 and NBLK == 4

    if dt_mode == "bf16":
        dt_x = bf16  # storage dtype of matmul inputs

        def mm(ap):
            return ap
    elif dt_mode == "fp16":
        dt_x = mybir.dt.float16

        def mm(ap):
            return ap
    else:
        dt_x = f32

        def mm(ap):
            return ap.bitcast(f32r)

    nc = bacc.Bacc(
        "TRN2", target_bir_lowering=False, debug=False, num_devices=N_CORES
    )

    xq = nc.dram_tensor("xq", [s, D], f32, kind="ExternalInput").ap()
    xk = nc.dram_tensor("xk", [s, D], f32, kind="ExternalInput").ap()
    xv = nc.dram_tensor("xv", [s, D], f32, kind="ExternalInput").ap()
    wq = nc.dram_tensor("wq", [D, C], f32, kind="ExternalInput").ap()
    wk = nc.dram_tensor("wk", [D, C], f32, kind="ExternalInput").ap()
    wv = nc.dram_tensor("wv", [D, C], f32, kind="ExternalInput").ap()
    bq = nc.dram_tensor("bq", [C], f32, kind="ExternalInput").ap()
    bk = nc.dram_tensor("bk", [C], f32, kind="ExternalInput").ap()
    bv = nc.dram_tensor("bv", [C], f32, kind="ExternalInput").ap()
    out = nc.dram_tensor("out", [s, C], f32, kind="ExternalOutput").ap()

    with tile.TileContext(nc) as tc:
        with (
            tc.tile_pool(name="const", bufs=1) as const_pool,
            tc.tile_pool(name="wts", bufs=1) as wts_pool,
            tc.tile_pool(name="qkv", bufs=1) as qkv_pool,
            tc.tile_pool(name="xn", bufs=4) as xn_pool,
            tc.tile_pool(name="xt", bufs=3) as xt_pool,
        ):
            ident = const_pool.tile([P, P], dt_x)
            make_identity(nc, ident[:])
            ident_f = const_pool.tile([P, P], f32)
            make_identity(nc, ident_f[:])

            # First x block's DMA goes on the queue before the weights so the
            # PE can start transposing ~6us in; weights follow (k first).
            xn_first = xn_pool.tile([P, 4, D], dt_x, tag="xn", name="xn_first")
            for t in range(4):
                nc.gpsimd.dma_start(
                    xn_first[:, t, :],
                    xk[t * P : (t + 1) * P, :].rearrange("(t p) d -> p t d", p=P)[
                        :, 0
                    ],
                )

            # weights: [p, dc, c] where d = dc*128 + p
            w_sb = {}
            for name, ap in (("k", wk), ("v", wv), ("q", wq)):
                t = wts_pool.tile([P, DB, C], dt_x, tag=f"w_{name}", name=f"w_{name}")
                nc.gpsimd.dma_start(t[:], ap.rearrange("(dc p) c -> p dc c", p=P))
                w_sb[name] = t
            # biases for q/k: [p, cc] with c = cc*128 + p
            b_sb = {}
            for name, ap in (("q", bq), ("k", bk)):
                t = const_pool.tile([P, CB], f32, tag=f"b_{name}", name=f"b_{name}")
                nc.sync.dma_start(t[:], ap.rearrange("(cc p) -> p cc", p=P))
                b_sb[name] = t
            # v bias as a row vector + ones row for the K=1 bias matmul
            bv_row = const_pool.tile([1, C], dt_x)
            nc.gpsimd.dma_start(bv_row[:], bv[None, :])
            ones_row = const_pool.tile([1, P], dt_x)
            nc.vector.memset(ones_row[:], 1.0)

            # projection outputs (persistent)
            qT = qkv_pool.tile([P, CB, s], dt_x)  # q^T: [c%128, c//128, s]
            kT = qkv_pool.tile([P, CB, s], dt_x)
            v1 = qkv_pool.tile([P, SB, NH, HD + 1], dt_x)  # [ki%128, ki//128, h, d|1]
            nc.vector.memset(v1[:, :, :, HD : HD + 1], 1.0)

            def emit_xn_dma(x_ap, blk):
                xn = xn_pool.tile([P, 4, D], dt_x, tag="xn")
                src = x_ap[blk * 512 : (blk + 1) * 512, :].rearrange(
                    "(t p) d -> p t d", p=P
                )
                for t in range(4):
                    nc.gpsimd.dma_start(xn[:, t, :], src[:, t, :])
                return xn

            def emit_qk_proj_cc(name, blk, xt, cc, pj_tile):
                dsttile = qT if name == "q" else kT
                for dc in range(DB):
                    nc.tensor.matmul(
                        pj_tile[:],
                        mm(w_sb[name][:, dc, cc * P : (cc + 1) * P]),
                        mm(xt[:, dc, :]),
                        start=(dc == 0),
                        stop=(dc == DB - 1),
                    )
                nc.vector.tensor_scalar_add(
                    dsttile[:, cc, blk * 512 : (blk + 1) * 512],
                    pj_tile[:],
                    b_sb[name][:, cc : cc + 1],
                )

            # ---------------- prefix: k, v, q-block-0 ----------------
            with (
                tc.tile_pool(name="ps_tr", bufs=2, space="PSUM") as ps_tr,
                tc.tile_pool(name="ps_pj", bufs=2, space="PSUM") as ps_pj,
                tc.tile_pool(name="ps_pv", bufs=2, space="PSUM") as ps_pv,
            ):
                n_evict = 0

                def emit_proj(name, blk, xt):
                    if name in ("q", "k"):
                        for cc in range(CB):
                            ps = ps_pj.tile([P, 512], f32, tag="pj")
                            emit_qk_proj_cc(name, blk, xt, cc, ps)
                    else:
                        for t in range(4):
                            sc = blk * 4 + t
                            ps = ps_pv.tile([P, C], f32, tag="pv")
                            for dc in range(DB):
                                nc.tensor.matmul(
                                    ps[:],
                                    mm(xt[:, dc, t * P : (t + 1) * P]),
                                    mm(w_sb["v"][:, dc, :]),
                                    start=(dc == 0),
                                    stop=False,
                                )
                            nc.tensor.matmul(
                                ps[:],
                                mm(ones_row[:, :]),
                                mm(bv_row[:, :]),
                                start=False,
                                stop=True,
                            )
                            nc.vector.tensor_copy(
                                v1[:, sc, :, 0:HD],
                                ps.rearrange("p (h e) -> p h e", h=NH),
                            )

                prefix_items = [("k", xk, blk) for blk in range(NBLK)]
                prefix_items += [("v", xv, blk) for blk in range(NBLK)]
                prefix_items += [
                    ("q", xq, blk) for blk in range(1 if OVERLAP_Q else NBLK)
                ]
                pending = None  # (name, blk, xt) with projections still to emit
                for name, x_ap, blk in prefix_items:
                    if name == "k" and blk == 0:
                        xn = xn_first
                    else:
                        xn = emit_xn_dma(x_ap, blk)
                    xt = xt_pool.tile([P, DB, 512], dt_x, tag="xt")
                    for t in range(4):
                        # 8 transposed chunks into one [128, 8, 128] psum
                        # tile, evicted with a single wide copy.
                        ps = ps_tr.tile([P, DB, P], f32, tag="tr")
                        for dc in range(DB):
                            nc.tensor.matmul(
                                ps[:, dc, :],
                                mm(xn[:, t, dc * P : (dc + 1) * P]),
                                mm(ident[:]),
                                start=True,
                                stop=True,
                            )
                        dst = xt.rearrange("p dc (t q) -> p t dc q", q=P)[:, t]
                        nc.vector.tensor_copy(dst, ps[:])
                        n_evict += 1
                    if pending is not None:
                        emit_proj(*pending)
                    pending = (name, blk, xt)
                emit_proj(*pending)

            # ---------------- attention stream ----------------
            with (
                tc.tile_pool(name="ps_sc", bufs=2, space="PSUM") as ps_sc,
                tc.tile_pool(name="ps_av", bufs=4, space="PSUM") as ps_av,
                tc.tile_pool(name="pexp", bufs=3) as p_pool,
                tc.tile_pool(name="osb", bufs=2) as o_pool,
                tc.tile_pool(name="outsb", bufs=2) as out_pool,
            ):

                avpack = os.environ.get("MHA_AVPACK", "1") == "1"

                def emit_av(hp, av, pex, ktp, last):
                    if not avpack:
                        for head in range(2):
                            nc.tensor.matmul(
                                av[head][:],
                                mm(v1[:, ktp, 2 * hp + head, :]),
                                mm(pex[:, head * QB : (head + 1) * QB]),
                                start=(ktp == 0),
                                stop=last,
                            )
                        return
                    # The K=128 AV contraction is split into two K=64 halves on
                    # distinct PE row groups (tile_position (0,0)/(64,0), auto-
                    # inferred from base partitions). Pairs (h0,half0)+(h1,half1)
                    # and (h0,half1)+(h1,half0) run concurrently on the array,
                    # halving the AV stream cost.
                    order = (
                        ((0, 0), (1, 1), (0, 1), (1, 0))
                        if os.environ.get("MHA_AVORD", "0") == "0"
                        else ((0, 0), (0, 1), (1, 1), (1, 0))
                    )
                    first = {}
                    lastmm = {}
                    for head, half in order:
                        first.setdefault(head, (head, half))
                        lastmm[head] = (head, half)
                    for head, half in order:
                        r0 = half * 64
                        nc.tensor.matmul(
                            av[head][:],
                            mm(v1[r0 : r0 + 64, ktp, 2 * hp + head, :]),
                            mm(pex[r0 : r0 + 64, head * QB : (head + 1) * QB]),
                            start=(ktp == 0 and first[head] == (head, half)),
                            stop=(last and lastmm[head] == (head, half)),
                        )

                def emit_tail_half(hp, qb, av, head, out_sb):
                    o_sb = o_pool.tile(
                        [HD + 1, QB], f32, tag="osb", name=f"osb{hp}_{qb}_{head}"
                    )
                    nc.vector.tensor_copy(o_sb[:], av[head][:])
                    tp = ps_av.tile(
                        [P, NJ, HD + 1],
                        f32,
                        tag="av",
                        name=f"tp{hp}_{qb}_{head}",
                    )
                    for j in range(NJ):
                        nc.tensor.transpose(
                            tp[:, j, :],
                            o_sb[:, j * P : (j + 1) * P],
                            ident_f[: HD + 1, : HD + 1],
                        )
                    rsb = o_pool.tile(
                        [P, NJ], f32, tag="rsb", name=f"rsb{hp}_{qb}_{head}"
                    )
                    nc.vector.reciprocal(rsb[:], tp[:, :, HD])
                    for j in range(NJ):
                        nc.vector.tensor_scalar_mul(
                            out_sb[:, j, head * HD : (head + 1) * HD],
                            tp[:, j, 0:HD],
                            rsb[:, j : j + 1],
                        )

                def emit_tail_dma(hp, qb, out_sb):
                    q0 = qb * QB
                    nc.sync.dma_start(
                        out[q0 : q0 + QB, hp * P : (hp + 1) * P].rearrange(
                            "(j p) c -> p j c", p=P
                        ),
                        out_sb[:],
                    )

                # woven q-block work: unit index -> q block to process
                qwork = {}
                if OVERLAP_Q:
                    for u, blk in enumerate(range(1, NBLK)):
                        qwork[u] = blk
                qstate = {}  # per live q block: dict(xn=, xt=, pj=)

                def emit_qwork(blk, kt):
                    st = qstate[blk]
                    if kt == 0:
                        st["xn"] = emit_xn_dma(xq, blk)
                        st["xt"] = xt_pool.tile(
                            [P, DB, 512], dt_x, tag="xt", name=f"xt_q{blk}"
                        )
                    elif 3 <= kt <= 6:
                        t = kt - 3
                        for dhalf in range(2):
                            tr = ps_av.tile(
                                [P, 4, P],
                                f32,
                                tag="av",
                                name=f"tr_q{blk}_{t}_{dhalf}",
                            )
                            for i in range(4):
                                dc = dhalf * 4 + i
                                nc.tensor.matmul(
                                    tr[:, i, :],
                                    mm(st["xn"][:, t, dc * P : (dc + 1) * P]),
                                    mm(ident[:]),
                                    start=True,
                                    stop=True,
                                )
                            nc.vector.tensor_copy(
                                st["xt"][
                                    :, dhalf * 4 : dhalf * 4 + 4, t * P : (t + 1) * P
                                ],
                                tr[:],
                            )
                    elif 7 <= kt <= 14:
                        cc, half = divmod(kt - 7, 4)
                        if half == 0:
                            st["pj"] = ps_av.tile(
                                [P, 512], f32, tag="av", name=f"pj_q{blk}_{cc}"
                            )
                        for dc in range(half * 2, half * 2 + 2):
                            nc.tensor.matmul(
                                st["pj"][:],
                                mm(w_sb["q"][:, dc, cc * P : (cc + 1) * P]),
                                mm(st["xt"][:, dc, :]),
                                start=(dc == 0),
                                stop=(dc == DB - 1),
                            )
                        if half == 3:
                            nc.vector.tensor_scalar_add(
                                qT[:, cc, blk * 512 : (blk + 1) * 512],
                                st["pj"][:],
                                b_sb["q"][:, cc : cc + 1],
                            )
                            del st["pj"]

                KT_A = max(1, SB // 8)
                KT_B = max(KT_A + 1, min(4, SB - 1))
                tail_prev = None  # (hp, qb, av) of the finished unit
                tail_outsb = None
                uidx = 0
                for hp in range(CB):  # head pair (c-chunk)
                    for qb in range(NQB):  # qi block of 512
                        q0 = qb * QB
                        if uidx in qwork:
                            qstate[qwork[uidx]] = {}
                        av = {}
                        for head in range(2):
                            av[head] = ps_av.tile(
                                [HD + 1, QB], f32, tag="av", name=f"av{hp}_{qb}_{head}"
                            )
                        # scores/exp stream one ki-tile ahead of the AV
                        # matmuls so the ACT exp stream never stalls on PE.
                        pex_q = []
                        for kt in range(SB):
                            sc_ps = ps_sc.tile([P, 2 * QB], f32, tag="sc")
                            for head in range(2):
                                r0 = head * HD
                                nc.tensor.matmul(
                                    sc_ps[:, head * QB : (head + 1) * QB],
                                    mm(kT[r0 : r0 + HD, hp, kt * P : (kt + 1) * P]),
                                    mm(qT[r0 : r0 + HD, hp, q0 : q0 + QB]),
                                    start=True,
                                    stop=True,
                                )
                            pex = p_pool.tile([P, 2 * QB], dt_x, tag="pex")
                            nc.scalar.activation(
                                pex[:], sc_ps[:], AF.Exp, bias=0.0, scale=0.125
                            )
                            pex_q.append(pex)
                            if kt >= 1:
                                emit_av(hp, av, pex_q[kt - 1], kt - 1, False)
                            if kt == KT_A and tail_prev is not None:
                                tail_outsb = out_pool.tile(
                                    [P, NJ, P],
                                    f32,
                                    tag="outsb",
                                    name=f"outsb{tail_prev[0]}_{tail_prev[1]}",
                                )
                                emit_tail_half(*tail_prev, 0, tail_outsb)
                            if kt == KT_B and tail_prev is not None:
                                emit_tail_half(*tail_prev, 1, tail_outsb)
                                emit_tail_dma(tail_prev[0], tail_prev[1], tail_outsb)
                                tail_prev = None
                            if uidx in qwork:
                                emit_qwork(qwork[uidx], kt)
                        emit_av(hp, av, pex_q[SB - 1], SB - 1, True)
                        tail_prev = (hp, qb, av)
                        uidx += 1
                tail_outsb = out_pool.tile(
                    [P, NJ, P], f32, tag="outsb", name="outsb_last"
                )
                emit_tail_half(*tail_prev, 0, tail_outsb)
                emit_tail_half(*tail_prev, 1, tail_outsb)
                emit_tail_dma(tail_prev[0], tail_prev[1], tail_outsb)
    nc.compile()
    return nc


def build_nc_v3(dt_mode: str = "fp16", s: int = S):
    """Sweep-structured kernel: kt-block-outer so the softmax exp stream (the
    ScalarE wall, ~147us) starts ~16us in and never starves.

    Stream = NBLK sweeps x NU units x KB kt-tiles. AV partials accumulate in
    PSUM within a sweep-visit and are folded into an SBUF accumulator between
    sweeps. All input-block production (DMA, PE transposes, projections) except
    (k0, q0) is woven into the stream's PE slack via a deadline-forced work
    queue. PSUM: 4 banks scores (double-buffered) + 2 AV + 1 transpose + 1
    projection.
    """
    assert s % 512 == 0
    SB = s // P
    NBLK = s // 512
    KB = SB // NBLK  # 4 kt per sweep visit
    QB = 512
    NQB = s // QB
    NU = NQB * CB  # units: u -> (qb, hp)
    NJ = QB // P

    if dt_mode == "bf16":
        dt_x = bf16

        def mm(ap):
            return ap
    elif dt_mode == "fp16":
        dt_x = mybir.dt.float16

        def mm(ap):
            return ap
    else:
        dt_x = f32

        def mm(ap):
            return ap.bitcast(f32r)

    nc = bacc.Bacc(
        "TRN2", target_bir_lowering=False, debug=False, num_devices=N_CORES
    )

    xq = nc.dram_tensor("xq", [s, D], f32, kind="ExternalInput").ap()
    xk = nc.dram_tensor("xk", [s, D], f32, kind="ExternalInput").ap()
    xv = nc.dram_tensor("xv", [s, D], f32, kind="ExternalInput").ap()
    wq = nc.dram_tensor("wq", [D, C], f32, kind="ExternalInput").ap()
    wk = nc.dram_tensor("wk", [D, C], f32, kind="ExternalInput").ap()
    wv = nc.dram_tensor("wv", [D, C], f32, kind="ExternalInput").ap()
    bq = nc.dram_tensor("bq", [C], f32, kind="ExternalInput").ap()
    bk = nc.dram_tensor("bk", [C], f32, kind="ExternalInput").ap()
    bv = nc.dram_tensor("bv", [C], f32, kind="ExternalInput").ap()
    out = nc.dram_tensor("out", [s, C], f32, kind="ExternalOutput").ap()
    x_aps = {"q": xq, "k": xk, "v": xv}
    w_aps = {"q": wq, "k": wk, "v": wv}

    with tile.TileContext(nc) as tc:
        with (
            tc.tile_pool(name="const", bufs=1) as const_pool,
            tc.tile_pool(name="wts", bufs=1) as wts_pool,
            tc.tile_pool(name="qkv", bufs=1) as qkv_pool,
            tc.tile_pool(name="xn", bufs=4) as xn_pool,
            tc.tile_pool(name="xt", bufs=2) as xt_pool,
            tc.tile_pool(name="pex", bufs=6) as pex_pool,
            tc.tile_pool(name="osb", bufs=2) as o_pool,
            tc.tile_pool(name="outsb", bufs=2) as out_pool,
            tc.tile_pool(name="ps_sc", bufs=2, space="PSUM") as ps_sc,
            tc.tile_pool(name="ps_av", bufs=2, space="PSUM") as ps_av,
            tc.tile_pool(name="ps_wk", bufs=2, space="PSUM") as ps_wk,
        ):
            ident = const_pool.tile([P, P], dt_x)
            make_identity(nc, ident[:])
            ident_f = const_pool.tile([P, P], f32)
            make_identity(nc, ident_f[:])
            ones_row = const_pool.tile([1, P], dt_x)
            nc.vector.memset(ones_row[:], 1.0)
            # warm the exp table set during the DMA-bound prefix
            warm = const_pool.tile([1, 8], f32)
            nc.vector.memset(warm[:], 0.0)
            warm2 = const_pool.tile([1, 8], dt_x)
            nc.scalar.activation(warm2[:], warm[:], AF.Exp, bias=0.0, scale=1.0)

            qT = qkv_pool.tile([P, CB, s], dt_x)
            kT = qkv_pool.tile([P, CB, s], dt_x)
            v1 = qkv_pool.tile([P, SB, NH, HD + 1], dt_x)
            nc.vector.memset(v1[:, :, :, HD : HD + 1], 1.0)
            o_acc = None
            if NBLK > 1:
                o_acc = qkv_pool.tile([HD + 1, NU, 2, QB], f32, name="o_acc")

            w_sb = {}
            b_sb = {}
            bv_row = const_pool.tile([1, C], dt_x)
            bstate = {}

            def emit_dma(name, blk):
                xn = xn_pool.tile([P, 4, D], dt_x, tag="xn", name=f"xn_{name}{blk}")
                src = x_aps[name][blk * 512 : (blk + 1) * 512, :].rearrange(
                    "(t p) d -> p t d", p=P
                )
                for t in range(4):
                    nc.gpsimd.dma_start(xn[:, t, :], src[:, t, :])
                bstate[(name, blk)]["xn"] = xn

            def emit_w(name):
                # f32 via the parallel HWDGE ring (keeps the SWDGE queue pure
                # x-blocks), cast on DVE which is idle during the prefix
                tf = wts_pool.tile(
                    [P, DB, C], f32, tag=f"wf_{name}", name=f"wf_{name}"
                )
                nc.sync.dma_start(
                    tf[:], w_aps[name].rearrange("(dc p) c -> p dc c", p=P)
                )
                t = wts_pool.tile([P, DB, C], dt_x, tag=f"w_{name}", name=f"w_{name}")
                nc.vector.tensor_copy(t[:], tf[:])
                w_sb[name] = t

            def emit_tr(name, blk, t, half):
                st = bstate[(name, blk)]
                if "xt" not in st:
                    st["xt"] = xt_pool.tile(
                        [P, DB, 512], dt_x, tag="xt", name=f"xt_{name}{blk}"
                    )
                ps = ps_wk.tile([P, 512], f32, tag="wk")
                psv = ps.rearrange("p (i q) -> p i q", q=P)
                for i in range(4):
                    dc = half * 4 + i
                    nc.tensor.matmul(
                        psv[:, i, :],
                        mm(st["xn"][:, t, dc * P : (dc + 1) * P]),
                        mm(ident[:]),
                        start=True,
                        stop=True,
                    )
                nc.vector.tensor_copy(
                    st["xt"][:, half * 4 : half * 4 + 4, t * P : (t + 1) * P],
                    psv[:],
                )

            def emit_pj(name, blk, cc, sh):
                # self-contained projection of a 256-row s-slice: the psum tile
                # lives only within this item, so ps_wk stays safe for
                # out-of-queue-order tail allocations
                st = bstate[(name, blk)]
                dsttile = qT if name == "q" else kT
                ps = ps_wk.tile([P, 512], f32, tag="wk", name=f"pj_{name}{blk}_{cc}_{sh}")
                s0 = sh * 256
                for dc in range(DB):
                    nc.tensor.matmul(
                        ps[:, 0:256],
                        mm(w_sb[name][:, dc, cc * P : (cc + 1) * P]),
                        mm(st["xt"][:, dc, s0 : s0 + 256]),
                        start=(dc == 0),
                        stop=(dc == DB - 1),
                    )
                nc.vector.tensor_scalar_add(
                    dsttile[:, cc, blk * 512 + s0 : blk * 512 + s0 + 256],
                    ps[:, 0:256],
                    b_sb[name][:, cc : cc + 1],
                )

            def emit_pv(blk, t):
                st = bstate[("v", blk)]
                sc = blk * 4 + t
                ps = ps_wk.tile([P, 512], f32, tag="wk")
                for dc in range(DB):
                    nc.tensor.matmul(
                        ps[:, 0:C],
                        mm(st["xt"][:, dc, t * P : (t + 1) * P]),
                        mm(w_sb["v"][:, dc, :]),
                        start=(dc == 0),
                        stop=False,
                    )
                nc.tensor.matmul(
                    ps[:, 0:C],
                    mm(ones_row[:, :]),
                    mm(bv_row[:, :]),
                    start=False,
                    stop=True,
                )
                nc.vector.tensor_copy(
                    v1[:, sc, :, 0:HD],
                    ps[:, 0:C].rearrange("p (h e) -> p h e", h=NH),
                )

            def block_items(name, blk):
                # items tagged with a drain sub-key; for v the sub-key is
                # per-kt so AV forces drain only what they need
                items = []
                if name in ("q", "k"):
                    key = (name, blk)
                    for t in range(4):
                        for half in range(2):
                            items.append(
                                (key, 0.45, (lambda n, b, tt, hh: lambda: emit_tr(n, b, tt, hh))(name, blk, t, half))
                            )
                    for cc in range(CB):
                        for sh in range(2):
                            items.append(
                                (key, 0.9, (lambda n, b, c, s_: lambda: emit_pj(n, b, c, s_))(name, blk, cc, sh))
                            )
                else:
                    for t in range(4):
                        key = ("v", blk, t)
                        for half in range(2):
                            items.append(
                                (key, 0.45, (lambda b, tt, hh: lambda: emit_tr("v", b, tt, hh))(blk, t, half))
                            )
                        items.append(
                            (key, 1.0, (lambda b, tt: lambda: emit_pv(b, tt))(blk, t))
                        )
                return items

            # ---------------- prefix ----------------
            for (name, blk) in [(n, b) for n in ("q", "k", "v") for b in range(NBLK)]:
                bstate[(name, blk)] = {}
            emit_dma("k", 0)
            emit_w("k")
            emit_dma("q", 0)
            emit_w("q")
            emit_dma("v", 0)
            emit_w("v")
            nc.gpsimd.dma_start(bv_row[:], bv[None, :])
            for name, ap in (("q", bq), ("k", bk)):
                t = const_pool.tile([P, CB], f32, tag=f"b_{name}", name=f"b_{name}")
                nc.sync.dma_start(t[:], ap.rearrange("(cc p) -> p cc", p=P))
                b_sb[name] = t
            for _, cost, fn in block_items("k", 0) + block_items("q", 0):
                fn()

            # ---------------- weave queue ----------------
            queue_blocks = [("v", 0)]
            queue_blocks += [("q", b) for b in range(1, NQB)]
            for b in range(1, NBLK):
                queue_blocks += [("k", b), ("v", b)]
            qitems = {key: block_items(*key) for key in queue_blocks}
            # DMA for block i leads by one queue position
            for i, key in enumerate(queue_blocks):
                lead = queue_blocks[max(0, i - 1)]
                dma_fn = (lambda k: lambda: emit_dma(*k))(key)
                qitems[lead].insert(0, (("dma",) + key, 0.15, dma_fn))
            qlist = [
                (subkey, cost, fn)
                for key in queue_blocks
                for subkey, cost, fn in qitems[key]
            ]
            qpos = [0]  # next index into qlist
            total_cost = sum(c for _, c, _ in qlist)
            done_upto = {}
            for i, (key, _, _) in enumerate(qlist):
                done_upto[key] = i + 1  # drain-through index per block

            def drain_through(key):
                tgt = done_upto.get(key, 0)
                while qpos[0] < tgt:
                    _, _, fn = qlist[qpos[0]]
                    fn()
                    qpos[0] += 1

            cum = [0.0]
            # deadline-aware drain plan: piecewise-linear cum-cost targets so
            # forced drains never burst (bursts block scores in the PE FIFO
            # and starve the exp stream)
            block_deadline = {}
            for key in queue_blocks:
                name, b = key
                if name == "v":
                    block_deadline[key] = max(2, b * NU * KB - 1)
                elif name == "q":
                    block_deadline[key] = max(2, b * CB * KB - 1)
                else:  # k
                    block_deadline[key] = max(2, b * NU * KB - 3)
            plan = []  # (slot, cum_cost_required)
            run = 0.0
            for key in queue_blocks:
                run += sum(c for _, c, _ in qitems[key])
                plan.append((block_deadline[key], run))
            plan.sort()

            def plan_target(slot):
                prev_s, prev_c = 0, 0.0
                for ds, dc in plan:
                    if slot < ds:
                        return prev_c + (dc - prev_c) * (slot + 1 - prev_s) / max(
                            1, ds - prev_s
                        )
                    prev_s, prev_c = ds, dc
                return total_cost

            def budget_pop(slot, n_slots, drain_slots):
                tgt = max(
                    plan_target(slot),
                    total_cost * min(1.0, (slot + 1) / max(1, drain_slots)),
                )
                while qpos[0] < len(qlist) and cum[0] < tgt:
                    _, c, fn = qlist[qpos[0]]
                    fn()
                    qpos[0] += 1
                    cum[0] += c

            # ---------------- stream ----------------
            avpack = os.environ.get("MHA_AVPACK", "0") == "1"

            def emit_av(hp, av, pex, ktp, first, last):
                if avpack:
                    for head, half in ((0, 0), (1, 1), (0, 1), (1, 0)):
                        r0 = half * 64
                        nc.tensor.matmul(
                            av[head][0 : HD + 1, :],
                            mm(v1[r0 : r0 + 64, ktp, 2 * hp + head, :]),
                            mm(pex[r0 : r0 + 64, head * QB : (head + 1) * QB]),
                            start=(first and half == head),
                            stop=(last and half != head),
                        )
                    return
                for head in range(2):
                    nc.tensor.matmul(
                        av[head][0 : HD + 1, :],
                        mm(v1[:, ktp, 2 * hp + head, :]),
                        mm(pex[:, head * QB : (head + 1) * QB]),
                        start=first,
                        stop=last,
                    )

            def emit_tail_p1(u, av):
                # softmax-fold phase: DVE combines the last sweep's PSUM
                # partials with the SBUF accumulator; runs at the flush so the
                # av pool slots free up immediately
                osbs = []
                for head in range(2):
                    o_sb = o_pool.tile(
                        [HD + 1, QB], f32, tag="osb", name=f"osb{u}_{head}"
                    )
                    if NBLK > 1:
                        nc.vector.scalar_tensor_tensor(
                            o_sb[:],
                            av[head][0 : HD + 1, :],
                            1.0,
                            o_acc[:, u, head, :],
                            op0=mybir.AluOpType.mult,
                            op1=mybir.AluOpType.add,
                        )
                    else:
                        nc.vector.tensor_copy(o_sb[:], av[head][0 : HD + 1, :])
                    osbs.append(o_sb)
                return osbs

            def emit_tail_p2(u, osbs):
                # transpose+normalize phase: emitted two slots later so the PE
                # transposes never wait on the p1 DVE adds in the FIFO
                qb, hp = divmod(u, CB)
                out_sb = out_pool.tile(
                    [P, NJ, P], f32, tag="outsb", name=f"outsb{u}"
                )
                for head in range(2):
                    tp = ps_wk.tile([P, 512], f32, tag="wk", name=f"tp{u}_{head}")
                    tpv = tp.rearrange("p (i q) -> p i q", q=P)
                    for j in range(NJ):
                        nc.tensor.transpose(
                            tpv[:, j, 0 : HD + 1],
                            osbs[head][:, j * P : (j + 1) * P],
                            ident_f[: HD + 1, : HD + 1],
                        )
                    rsb = o_pool.tile([P, NJ], f32, tag="rsb", name=f"rsb{u}_{head}")
                    nc.vector.reciprocal(rsb[:], tpv[:, :, HD])
                    for j in range(NJ):
                        nc.vector.tensor_scalar_mul(
                            out_sb[:, j, head * HD : (head + 1) * HD],
                            tpv[:, j, 0:HD],
                            rsb[:, j : j + 1],
                        )
                q0r = qb * QB
                nc.sync.dma_start(
                    out[q0r : q0r + QB, hp * P : (hp + 1) * P].rearrange(
                        "(j p) c -> p j c", p=P
                    ),
                    out_sb[:],
                )

            n_slots = NBLK * NU * KB
            drain_slots = max(1, n_slots - NU * KB)
            pend = None  # (u, hp, av, pex, ktp, kb)
            tail2 = None  # (u, osbs) awaiting phase-2
            slot = 0
            for kb in range(NBLK):
                for u in range(NU):
                    qb, hp = divmod(u, CB)
                    drain_through(("k", kb))
                    drain_through(("q", qb))
                    av = None
                    q0r = qb * QB
                    pexs = {}
                    for j in range(KB):
                        kt = kb * KB + j
                        sc_ps = ps_sc.tile([P, 2 * QB], f32, tag="sc")
                        for head in range(2):
                            r0 = head * HD
                            nc.tensor.matmul(
                                sc_ps[:, head * QB : (head + 1) * QB],
                                mm(kT[r0 : r0 + HD, hp, kt * P : (kt + 1) * P]),
                                mm(qT[r0 : r0 + HD, hp, q0r : q0r + QB]),
                                start=True,
                                stop=True,
                            )
                        pex = pex_pool.tile([P, 2 * QB], dt_x, tag="pex")
                        nc.scalar.activation(
                            pex[:], sc_ps[:], AF.Exp, bias=0.0, scale=0.125
                        )
                        pexs[j] = pex
                        if j == 2 and tail2 is not None:
                            emit_tail_p2(*tail2)
                            tail2 = None
                        if j == 0:
                            if pend is not None:
                                pu, php, pav, ppex, pktp, pkb = pend
                                drain_through(("v", pktp // KB, pktp % KB))
                                emit_av(php, pav, ppex, pktp, False, True)
                                if pkb == NBLK - 1:
                                    # p1 must run before this visit's av tiles
                                    # reuse the pool slots
                                    tail2 = (pu, emit_tail_p1(pu, pav))
                                elif pkb == 0 and NBLK > 1:
                                    for h in range(2):
                                        nc.vector.tensor_copy(
                                            o_acc[:, pu, h, :],
                                            pav[h][0 : HD + 1, :],
                                        )
                                else:
                                    for h in range(2):
                                        nc.vector.scalar_tensor_tensor(
                                            o_acc[:, pu, h, :],
                                            pav[h][0 : HD + 1, :],
                                            1.0,
                                            o_acc[:, pu, h, :],
                                            op0=mybir.AluOpType.mult,
                                            op1=mybir.AluOpType.add,
                                        )
                                pend = None
                            av = {
                                h: ps_av.tile(
                                    [P, 512],
                                    f32,
                                    tag="av",
                                    name=f"av{kb}_{u}_{h}",
                                )
                                for h in range(2)
                            }
                        else:
                            ktp = kt - 1
                            drain_through(("v", ktp // KB, ktp % KB))
                            emit_av(hp, av, pexs[j - 1], ktp, j - 1 == 0, False)
                        budget_pop(slot, n_slots, drain_slots)
                        slot += 1
                    pend = (u, hp, av, pexs[KB - 1], kb * KB + KB - 1, kb)
            # final flush
            pu, php, pav, ppex, pktp, pkb = pend
            drain_through(("v", pktp // KB, pktp % KB))
            emit_av(php, pav, ppex, pktp, KB == 1, True)
            if tail2 is not None:
                emit_tail_p2(*tail2)
            emit_tail_p2(pu, emit_tail_p1(pu, pav))
    nc.compile()
    return nc


_CACHE = {}


def _get_nc(dt_mode: str):
    key = (dt_mode, os.environ.get("MHA_V2", "0"))
    if key not in _CACHE:
        if key[1] == "1":
            _CACHE[key] = build_nc(dt_mode)
        else:
            _CACHE[key] = build_nc_v3(dt_mode)
    return _CACHE[key]


def kernel(query, key, value, Wq, bq, Wk, bk, Wv, bv, **kwargs):
    _install_ntff_hook_shim()
    from concourse.bass_utils import run_bass_kernel_spmd

    dt_mode = os.environ.get("MHA_DT", "fp16")
    nc = _get_nc(dt_mode)

    query = np.asarray(query, dtype=np.float32)
    key = np.asarray(key, dtype=np.float32)
    value = np.asarray(value, dtype=np.float32)
    Wq = np.asarray(Wq, dtype=np.float32)
    Wk = np.asarray(Wk, dtype=np.float32)
    Wv = np.asarray(Wv, dtype=np.float32)
    bq = np.asarray(bq, dtype=np.float32)
    bk = np.asarray(bk, dtype=np.float32)
    bv = np.asarray(bv, dtype=np.float32)

    in_maps = []
    for c in range(N_CORES):
        b, g = divmod(c, GROUPS)
        cs = g * C
        in_maps.append(
            {
                "xq": np.ascontiguousarray(query[b]),
                "xk": np.ascontiguousarray(key[b]),
                "xv": np.ascontiguousarray(value[b]),
                "wq": np.ascontiguousarray(Wq[:, cs : cs + C]),
                "wk": np.ascontiguousarray(Wk[:, cs : cs + C]),
                "wv": np.ascontiguousarray(Wv[:, cs : cs + C]),
                "bq": np.ascontiguousarray(bq[cs : cs + C]),
                "bk": np.ascontiguousarray(bk[cs : cs + C]),
                "bv": np.ascontiguousarray(bv[cs : cs + C]),
            }
        )

    res = run_bass_kernel_spmd(
        nc, in_maps, core_ids=list(range(N_CORES)), **kwargs
    )
    outp = np.empty((B, S, D), dtype=np.float32)
    for c in range(N_CORES):
        b, g = divmod(c, GROUPS)
        outp[b, :, g * C : (g + 1) * C] = res.results[c]["out"]
    if kwargs:
        return outp, res
    return outp



# revision 39
# speedup vs baseline: 1.0492x; 1.0446x over previous
"""Multi-head attention (B=2, S=2048, D=1024, H=16) on 8 trn2 NeuronCores.

Sharding: 2-way over batch x 4-way over head groups (4 heads / 256 cols per
core). No cross-core communication.

Per-core kernel (Tile):
  prefix:  load X_k, X_v and the first X_q block (cast f32->bf16 during DMA),
           transpose 128x128 chunks via regular matmul against identity
           (counts as PE activity so the HAM clock gate stays open), project
           kT [256, 2048] (head dim on partitions), v natural [2048, 256]
           stored as [v | 1] per ki-tile (the ones column makes the AV matmul
           also emit softmax row-sums), and qT for block 0.
  stream:  per (head-pair, qi-block of 512): for each ki-tile: S^T = kT.T@qT
           with the two heads row-packed on the PE (K=64 each) into one
           [128, 1024] psum tile (bufs=2), one exp ACTIVATE per ki-tile
           (scale=1/8 folded in), AV matmuls one ki-tile behind the scores so
           the exp stream never stalls. The remaining X_q blocks are loaded /
           transposed / projected in small slices woven into the first three
           units' ki-loops (PSUM slots shared with the AV pool). Unit
           epilogues (out^T -> out transpose + softmax normalize) are split
           in half and woven into the next unit's ki-loop the same way.
"""

import os
import sys

import numpy as np

import concourse.bass as bass
import concourse.tile as tile
from concourse import bacc, mybir
from concourse.masks import make_identity

B, S, D = 2, 2048, 1024
H, HD = 16, 64
N_CORES = 8
GROUPS = 4  # head groups (cores per batch)
NH = H // GROUPS  # local heads per core = 4
C = NH * HD  # local output cols = 256
P = 128
DB = D // P  # 8 d-chunks
CB = C // P  # 2 c-chunks (head pairs)

f32 = mybir.dt.float32
bf16 = mybir.dt.bfloat16
f32r = mybir.dt.float32r

AF = mybir.ActivationFunctionType


def _install_ntff_hook_shim():
    """Best-effort: register the axon NTFF profile hook so a traced run
    (e.g. BASS_TRACE=1) works even when the image's antenv lacks axon_hooks."""
    try:
        import antenv.axon_hooks  # noqa: F401

        return
    except ImportError:
        pass
    try:
        import types

        _hook = [None]
        mod = types.ModuleType("antenv.axon_hooks")
        mod.set_axon_ntff_profile_hook = lambda h: _hook.__setitem__(0, h)
        mod.get_axon_ntff_profile_hook = lambda: _hook[0]
        sys.modules["antenv.axon_hooks"] = mod
        from trn_agent_boot.trn_boot import _ntff_profile_via_ctypes

        so = "/opt/axon/libaxon_pjrt.so"
        if os.path.exists(so):
            mod.set_axon_ntff_profile_hook(_ntff_profile_via_ctypes(so))
    except Exception:
        pass


def build_nc(dt_mode: str = "fp16", s: int = S):
    """Trace + compile the per-core Bass kernel. dt_mode in {"fp16", "bf16", "f32r"}."""
    assert s % 512 == 0
    SB = s // P  # ki-tiles
    NBLK = s // 512  # 512-row s-blocks
    QB = 512  # qi-block
    NQB = s // QB
    NJ = QB // P  # 128-chunks per qi-block = 4
    # overlap q blocks 1.. with the attention stream only at full size
    OVERLAP_Q = SB >= 16 and NBLK == 4

    if dt_mode == "bf16":
        dt_x = bf16  # storage dtype of matmul inputs

        def mm(ap):
            return ap
    elif dt_mode == "fp16":
        dt_x = mybir.dt.float16

        def mm(ap):
            return ap
    else:
        dt_x = f32

        def mm(ap):
            return ap.bitcast(f32r)

    nc = bacc.Bacc(
        "TRN2", target_bir_lowering=False, debug=False, num_devices=N_CORES
    )

    xq = nc.dram_tensor("xq", [s, D], f32, kind="ExternalInput").ap()
    xk = nc.dram_tensor("xk", [s, D], f32, kind="ExternalInput").ap()
    xv = nc.dram_tensor("xv", [s, D], f32, kind="ExternalInput").ap()
    wq = nc.dram_tensor("wq", [D, C], f32, kind="ExternalInput").ap()
    wk = nc.dram_tensor("wk", [D, C], f32, kind="ExternalInput").ap()
    wv = nc.dram_tensor("wv", [D, C], f32, kind="ExternalInput").ap()
    bq = nc.dram_tensor("bq", [C], f32, kind="ExternalInput").ap()
    bk = nc.dram_tensor("bk", [C], f32, kind="ExternalInput").ap()
    bv = nc.dram_tensor("bv", [C], f32, kind="ExternalInput").ap()
    out = nc.dram_tensor("out", [s, C], f32, kind="ExternalOutput").ap()

    with tile.TileContext(nc) as tc:
        with (
            tc.tile_pool(name="const", bufs=1) as const_pool,
            tc.tile_pool(name="wts", bufs=1) as wts_pool,
            tc.tile_pool(name="qkv", bufs=1) as qkv_pool,
            tc.tile_pool(name="xn", bufs=4) as xn_pool,
            tc.tile_pool(name="xt", bufs=3) as xt_pool,
        ):
            ident = const_pool.tile([P, P], dt_x)
            make_identity(nc, ident[:])
            ident_f = const_pool.tile([P, P], f32)
            make_identity(nc, ident_f[:])

            # First x block's DMA goes on the queue before the weights so the
            # PE can start transposing ~6us in; weights follow (k first).
            xn_first = xn_pool.tile([P, 4, D], dt_x, tag="xn", name="xn_first")
            for t in range(4):
                nc.gpsimd.dma_start(
                    xn_first[:, t, :],
                    xk[t * P : (t + 1) * P, :].rearrange("(t p) d -> p t d", p=P)[
                        :, 0
                    ],
                )

            # weights: [p, dc, c] where d = dc*128 + p
            w_sb = {}
            for name, ap in (("k", wk), ("v", wv), ("q", wq)):
                t = wts_pool.tile([P, DB, C], dt_x, tag=f"w_{name}", name=f"w_{name}")
                nc.gpsimd.dma_start(t[:], ap.rearrange("(dc p) c -> p dc c", p=P))
                w_sb[name] = t
            # biases for q/k: [p, cc] with c = cc*128 + p
            b_sb = {}
            for name, ap in (("q", bq), ("k", bk)):
                t = const_pool.tile([P, CB], f32, tag=f"b_{name}", name=f"b_{name}")
                nc.sync.dma_start(t[:], ap.rearrange("(cc p) -> p cc", p=P))
                b_sb[name] = t
            # v bias as a row vector + ones row for the K=1 bias matmul
            bv_row = const_pool.tile([1, C], dt_x)
            nc.gpsimd.dma_start(bv_row[:], bv[None, :])
            ones_row = const_pool.tile([1, P], dt_x)
            nc.vector.memset(ones_row[:], 1.0)

            # projection outputs (persistent)
            qT = qkv_pool.tile([P, CB, s], dt_x)  # q^T: [c%128, c//128, s]
            kT = qkv_pool.tile([P, CB, s], dt_x)
            v1 = qkv_pool.tile([P, SB, NH, HD + 1], dt_x)  # [ki%128, ki//128, h, d|1]
            nc.vector.memset(v1[:, :, :, HD : HD + 1], 1.0)

            def emit_xn_dma(x_ap, blk):
                xn = xn_pool.tile([P, 4, D], dt_x, tag="xn")
                src = x_ap[blk * 512 : (blk + 1) * 512, :].rearrange(
                    "(t p) d -> p t d", p=P
                )
                for t in range(4):
                    nc.gpsimd.dma_start(xn[:, t, :], src[:, t, :])
                return xn

            def emit_qk_proj_cc(name, blk, xt, cc, pj_tile):
                dsttile = qT if name == "q" else kT
                for dc in range(DB):
                    nc.tensor.matmul(
                        pj_tile[:],
                        mm(w_sb[name][:, dc, cc * P : (cc + 1) * P]),
                        mm(xt[:, dc, :]),
                        start=(dc == 0),
                        stop=(dc == DB - 1),
                    )
                nc.vector.tensor_scalar_add(
                    dsttile[:, cc, blk * 512 : (blk + 1) * 512],
                    pj_tile[:],
                    b_sb[name][:, cc : cc + 1],
                )

            # ---------------- prefix: k, v, q-block-0 ----------------
            with (
                tc.tile_pool(name="ps_tr", bufs=2, space="PSUM") as ps_tr,
                tc.tile_pool(name="ps_pj", bufs=2, space="PSUM") as ps_pj,
                tc.tile_pool(name="ps_pv", bufs=2, space="PSUM") as ps_pv,
            ):
                n_evict = 0

                def emit_proj(name, blk, xt):
                    if name in ("q", "k"):
                        for cc in range(CB):
                            ps = ps_pj.tile([P, 512], f32, tag="pj")
                            emit_qk_proj_cc(name, blk, xt, cc, ps)
                    else:
                        for t in range(4):
                            sc = blk * 4 + t
                            ps = ps_pv.tile([P, C], f32, tag="pv")
                            for dc in range(DB):
                                nc.tensor.matmul(
                                    ps[:],
                                    mm(xt[:, dc, t * P : (t + 1) * P]),
                                    mm(w_sb["v"][:, dc, :]),
                                    start=(dc == 0),
                                    stop=False,
                                )
                            nc.tensor.matmul(
                                ps[:],
                                mm(ones_row[:, :]),
                                mm(bv_row[:, :]),
                                start=False,
                                stop=True,
                            )
                            nc.vector.tensor_copy(
                                v1[:, sc, :, 0:HD],
                                ps.rearrange("p (h e) -> p h e", h=NH),
                            )

                prefix_items = [("k", xk, blk) for blk in range(NBLK)]
                prefix_items += [("v", xv, blk) for blk in range(NBLK)]
                prefix_items += [
                    ("q", xq, blk) for blk in range(1 if OVERLAP_Q else NBLK)
                ]
                pending = None  # (name, blk, xt) with projections still to emit
                for name, x_ap, blk in prefix_items:
                    if name == "k" and blk == 0:
                        xn = xn_first
                    else:
                        xn = emit_xn_dma(x_ap, blk)
                    xt = xt_pool.tile([P, DB, 512], dt_x, tag="xt")
                    for t in range(4):
                        # 8 transposed chunks into one [128, 8, 128] psum
                        # tile, evicted with a single wide copy.
                        ps = ps_tr.tile([P, DB, P], f32, tag="tr")
                        for dc in range(DB):
                            nc.tensor.matmul(
                                ps[:, dc, :],
                                mm(xn[:, t, dc * P : (dc + 1) * P]),
                                mm(ident[:]),
                                start=True,
                                stop=True,
                            )
                        dst = xt.rearrange("p dc (t q) -> p t dc q", q=P)[:, t]
                        nc.vector.tensor_copy(dst, ps[:])
                        n_evict += 1
                    if pending is not None:
                        emit_proj(*pending)
                    pending = (name, blk, xt)
                emit_proj(*pending)

            # ---------------- attention stream ----------------
            with (
                tc.tile_pool(name="ps_sc", bufs=2, space="PSUM") as ps_sc,
                tc.tile_pool(name="ps_av", bufs=4, space="PSUM") as ps_av,
                tc.tile_pool(name="pexp", bufs=3) as p_pool,
                tc.tile_pool(name="osb", bufs=2) as o_pool,
                tc.tile_pool(name="outsb", bufs=2) as out_pool,
            ):

                avpack = os.environ.get("MHA_AVPACK", "1") == "1"

                def emit_av(hp, av, pex, ktp, last):
                    if not avpack:
                        for head in range(2):
                            nc.tensor.matmul(
                                av[head][:],
                                mm(v1[:, ktp, 2 * hp + head, :]),
                                mm(pex[:, head * QB : (head + 1) * QB]),
                                start=(ktp == 0),
                                stop=last,
                            )
                        return
                    # The K=128 AV contraction is split into two K=64 halves on
                    # distinct PE row groups (tile_position (0,0)/(64,0), auto-
                    # inferred from base partitions). Pairs (h0,half0)+(h1,half1)
                    # and (h0,half1)+(h1,half0) run concurrently on the array,
                    # halving the AV stream cost.
                    order = (
                        ((0, 0), (1, 1), (0, 1), (1, 0))
                        if os.environ.get("MHA_AVORD", "0") == "0"
                        else ((0, 0), (0, 1), (1, 1), (1, 0))
                    )
                    first = {}
                    lastmm = {}
                    for head, half in order:
                        first.setdefault(head, (head, half))
                        lastmm[head] = (head, half)
                    for head, half in order:
                        r0 = half * 64
                        nc.tensor.matmul(
                            av[head][:],
                            mm(v1[r0 : r0 + 64, ktp, 2 * hp + head, :]),
                            mm(pex[r0 : r0 + 64, head * QB : (head + 1) * QB]),
                            start=(ktp == 0 and first[head] == (head, half)),
                            stop=(last and lastmm[head] == (head, half)),
                        )

                def emit_tail_half(hp, qb, av, head, out_sb):
                    o_sb = o_pool.tile(
                        [HD + 1, QB], f32, tag="osb", name=f"osb{hp}_{qb}_{head}"
                    )
                    nc.vector.tensor_copy(o_sb[:], av[head][:])
                    tp = ps_av.tile(
                        [P, NJ, HD + 1],
                        f32,
                        tag="av",
                        name=f"tp{hp}_{qb}_{head}",
                    )
                    for j in range(NJ):
                        nc.tensor.transpose(
                            tp[:, j, :],
                            o_sb[:, j * P : (j + 1) * P],
                            ident_f[: HD + 1, : HD + 1],
                        )
                    rsb = o_pool.tile(
                        [P, NJ], f32, tag="rsb", name=f"rsb{hp}_{qb}_{head}"
                    )
                    nc.vector.reciprocal(rsb[:], tp[:, :, HD])
                    for j in range(NJ):
                        nc.vector.tensor_scalar_mul(
                            out_sb[:, j, head * HD : (head + 1) * HD],
                            tp[:, j, 0:HD],
                            rsb[:, j : j + 1],
                        )

                def emit_tail_dma(hp, qb, out_sb):
                    q0 = qb * QB
                    nc.sync.dma_start(
                        out[q0 : q0 + QB, hp * P : (hp + 1) * P].rearrange(
                            "(j p) c -> p j c", p=P
                        ),
                        out_sb[:],
                    )

                # woven q-block work: unit index -> q block to process
                qwork = {}
                if OVERLAP_Q:
                    for u, blk in enumerate(range(1, NBLK)):
                        qwork[u] = blk
                qstate = {}  # per live q block: dict(xn=, xt=, pj=)

                def emit_qwork(blk, kt):
                    st = qstate[blk]
                    if kt == 0:
                        st["xn"] = emit_xn_dma(xq, blk)
                        st["xt"] = xt_pool.tile(
                            [P, DB, 512], dt_x, tag="xt", name=f"xt_q{blk}"
                        )
                    elif 3 <= kt <= 6:
                        t = kt - 3
                        for dhalf in range(2):
                            tr = ps_av.tile(
                                [P, 4, P],
                                f32,
                                tag="av",
                                name=f"tr_q{blk}_{t}_{dhalf}",
                            )
                            for i in range(4):
                                dc = dhalf * 4 + i
                                nc.tensor.matmul(
                                    tr[:, i, :],
                                    mm(st["xn"][:, t, dc * P : (dc + 1) * P]),
                                    mm(ident[:]),
                                    start=True,
                                    stop=True,
                                )
                            nc.vector.tensor_copy(
                                st["xt"][
                                    :, dhalf * 4 : dhalf * 4 + 4, t * P : (t + 1) * P
                                ],
                                tr[:],
                            )
                    elif 7 <= kt <= 14:
                        cc, half = divmod(kt - 7, 4)
                        if half == 0:
                            st["pj"] = ps_av.tile(
                                [P, 512], f32, tag="av", name=f"pj_q{blk}_{cc}"
                            )
                        for dc in range(half * 2, half * 2 + 2):
                            nc.tensor.matmul(
                                st["pj"][:],
                                mm(w_sb["q"][:, dc, cc * P : (cc + 1) * P]),
                                mm(st["xt"][:, dc, :]),
                                start=(dc == 0),
                                stop=(dc == DB - 1),
                            )
                        if half == 3:
                            nc.vector.tensor_scalar_add(
                                qT[:, cc, blk * 512 : (blk + 1) * 512],
                                st["pj"][:],
                                b_sb["q"][:, cc : cc + 1],
                            )
                            del st["pj"]

                KT_A = max(1, SB // 8)
                KT_B = max(KT_A + 1, min(4, SB - 1))
                tail_prev = None  # (hp, qb, av) of the finished unit
                tail_outsb = None
                uidx = 0
                for hp in range(CB):  # head pair (c-chunk)
                    for qb in range(NQB):  # qi block of 512
                        q0 = qb * QB
                        if uidx in qwork:
                            qstate[qwork[uidx]] = {}
                        av = {}
                        for head in range(2):
                            av[head] = ps_av.tile(
                                [HD + 1, QB], f32, tag="av", name=f"av{hp}_{qb}_{head}"
                            )
                        # scores/exp stream one ki-tile ahead of the AV
                        # matmuls so the ACT exp stream never stalls on PE.
                        pex_q = []
                        for kt in range(SB):
                            sc_ps = ps_sc.tile([P, 2 * QB], f32, tag="sc")
                            for head in range(2):
                                r0 = head * HD
                                nc.tensor.matmul(
                                    sc_ps[:, head * QB : (head + 1) * QB],
                                    mm(kT[r0 : r0 + HD, hp, kt * P : (kt + 1) * P]),
                                    mm(qT[r0 : r0 + HD, hp, q0 : q0 + QB]),
                                    start=True,
                                    stop=True,
                                )
                            pex = p_pool.tile([P, 2 * QB], dt_x, tag="pex")
                            nc.scalar.activation(
                                pex[:], sc_ps[:], AF.Exp, bias=0.0, scale=0.125
                            )
                            pex_q.append(pex)
                            if kt >= 1:
                                emit_av(hp, av, pex_q[kt - 1], kt - 1, False)
                            if kt == KT_A and tail_prev is not None:
                                tail_outsb = out_pool.tile(
                                    [P, NJ, P],
                                    f32,
                                    tag="outsb",
                                    name=f"outsb{tail_prev[0]}_{tail_prev[1]}",
                                )
                                emit_tail_half(*tail_prev, 0, tail_outsb)
                            if kt == KT_B and tail_prev is not None:
                                emit_tail_half(*tail_prev, 1, tail_outsb)
                                emit_tail_dma(tail_prev[0], tail_prev[1], tail_outsb)
                                tail_prev = None
                            if uidx in qwork:
                                emit_qwork(qwork[uidx], kt)
                        emit_av(hp, av, pex_q[SB - 1], SB - 1, True)
                        tail_prev = (hp, qb, av)
                        uidx += 1
                tail_outsb = out_pool.tile(
                    [P, NJ, P], f32, tag="outsb", name="outsb_last"
                )
                emit_tail_half(*tail_prev, 0, tail_outsb)
                emit_tail_half(*tail_prev, 1, tail_outsb)
                emit_tail_dma(tail_prev[0], tail_prev[1], tail_outsb)
    nc.compile()
    return nc


def build_nc_v3(dt_mode: str = "fp16", s: int = S):
    """Sweep-structured kernel: kt-block-outer so the softmax exp stream (the
    ScalarE wall, ~147us) starts ~16us in and never starves.

    Stream = NBLK sweeps x NU units x KB kt-tiles. AV partials accumulate in
    PSUM within a sweep-visit and are folded into an SBUF accumulator between
    sweeps. All input-block production (DMA, PE transposes, projections) except
    (k0, q0) is woven into the stream's PE slack via a deadline-forced work
    queue. PSUM: 4 banks scores (double-buffered) + 2 AV + 1 transpose + 1
    projection.
    """
    assert s % 512 == 0
    SB = s // P
    NBLK = s // 512
    KB = SB // NBLK  # 4 kt per sweep visit
    QB = 512
    NQB = s // QB
    NU = NQB * CB  # units: u -> (qb, hp)
    NJ = QB // P

    if dt_mode == "bf16":
        dt_x = bf16

        def mm(ap):
            return ap
    elif dt_mode == "fp16":
        dt_x = mybir.dt.float16

        def mm(ap):
            return ap
    else:
        dt_x = f32

        def mm(ap):
            return ap.bitcast(f32r)

    nc = bacc.Bacc(
        "TRN2", target_bir_lowering=False, debug=False, num_devices=N_CORES
    )

    xq = nc.dram_tensor("xq", [s, D], f32, kind="ExternalInput").ap()
    xk = nc.dram_tensor("xk", [s, D], f32, kind="ExternalInput").ap()
    xv = nc.dram_tensor("xv", [s, D], f32, kind="ExternalInput").ap()
    wq = nc.dram_tensor("wq", [D, C], f32, kind="ExternalInput").ap()
    wk = nc.dram_tensor("wk", [D, C], f32, kind="ExternalInput").ap()
    wv = nc.dram_tensor("wv", [D, C], f32, kind="ExternalInput").ap()
    bq = nc.dram_tensor("bq", [C], f32, kind="ExternalInput").ap()
    bk = nc.dram_tensor("bk", [C], f32, kind="ExternalInput").ap()
    bv = nc.dram_tensor("bv", [C], f32, kind="ExternalInput").ap()
    out = nc.dram_tensor("out", [s, C], f32, kind="ExternalOutput").ap()
    x_aps = {"q": xq, "k": xk, "v": xv}
    w_aps = {"q": wq, "k": wk, "v": wv}

    with tile.TileContext(nc) as tc:
        with (
            tc.tile_pool(name="const", bufs=1) as const_pool,
            tc.tile_pool(name="wts", bufs=1) as wts_pool,
            tc.tile_pool(name="qkv", bufs=1) as qkv_pool,
            tc.tile_pool(name="xn", bufs=4) as xn_pool,
            tc.tile_pool(name="xt", bufs=2) as xt_pool,
            tc.tile_pool(name="pex", bufs=8) as pex_pool,
            tc.tile_pool(name="osb", bufs=2) as o_pool,
            tc.tile_pool(name="outsb", bufs=2) as out_pool,
            tc.tile_pool(name="ps_sc", bufs=2, space="PSUM") as ps_sc,
            tc.tile_pool(name="ps_av", bufs=2, space="PSUM") as ps_av,
            tc.tile_pool(name="ps_wk", bufs=2, space="PSUM") as ps_wk,
        ):
            ident = const_pool.tile([P, P], dt_x)
            make_identity(nc, ident[:])
            ident_f = const_pool.tile([P, P], f32)
            make_identity(nc, ident_f[:])
            ones_row = const_pool.tile([1, P], dt_x)
            nc.vector.memset(ones_row[:], 1.0)
            # warm the exp table set during the DMA-bound prefix
            warm = const_pool.tile([1, 8], f32)
            nc.vector.memset(warm[:], 0.0)
            warm2 = const_pool.tile([1, 8], dt_x)
            nc.scalar.activation(warm2[:], warm[:], AF.Exp, bias=0.0, scale=1.0)

            qT = qkv_pool.tile([P, CB, s], dt_x)
            kT = qkv_pool.tile([P, CB, s], dt_x)
            v1 = qkv_pool.tile([P, SB, NH, HD + 1], dt_x)
            nc.vector.memset(v1[:, :, :, HD : HD + 1], 1.0)
            o_acc = None
            if NBLK > 1:
                o_acc = qkv_pool.tile([HD + 1, NU, 2, QB], f32, name="o_acc")

            w_sb = {}
            b_sb = {}
            bv_row = const_pool.tile([1, C], dt_x)
            bstate = {}

            def emit_dma(name, blk):
                xn = xn_pool.tile([P, 4, D], dt_x, tag="xn", name=f"xn_{name}{blk}")
                src = x_aps[name][blk * 512 : (blk + 1) * 512, :].rearrange(
                    "(t p) d -> p t d", p=P
                )
                for t in range(4):
                    nc.gpsimd.dma_start(xn[:, t, :], src[:, t, :])
                bstate[(name, blk)]["xn"] = xn

            def emit_w(name):
                t = wts_pool.tile([P, DB, C], dt_x, tag=f"w_{name}", name=f"w_{name}")
                nc.gpsimd.dma_start(
                    t[:], w_aps[name].rearrange("(dc p) c -> p dc c", p=P)
                )
                w_sb[name] = t

            def emit_tr(name, blk, t, half):
                st = bstate[(name, blk)]
                if "xt" not in st:
                    st["xt"] = xt_pool.tile(
                        [P, DB, 512], dt_x, tag="xt", name=f"xt_{name}{blk}"
                    )
                ps = ps_wk.tile([P, 512], f32, tag="wk")
                psv = ps.rearrange("p (i q) -> p i q", q=P)
                for i in range(4):
                    dc = half * 4 + i
                    nc.tensor.matmul(
                        psv[:, i, :],
                        mm(st["xn"][:, t, dc * P : (dc + 1) * P]),
                        mm(ident[:]),
                        start=True,
                        stop=True,
                    )
                nc.vector.tensor_copy(
                    st["xt"][:, half * 4 : half * 4 + 4, t * P : (t + 1) * P],
                    psv[:],
                )

            def emit_pj(name, blk, cc, sh):
                # self-contained projection of a 256-row s-slice: the psum tile
                # lives only within this item, so ps_wk stays safe for
                # out-of-queue-order tail allocations
                st = bstate[(name, blk)]
                dsttile = qT if name == "q" else kT
                ps = ps_wk.tile([P, 512], f32, tag="wk", name=f"pj_{name}{blk}_{cc}_{sh}")
                s0 = sh * 256
                for dc in range(DB):
                    nc.tensor.matmul(
                        ps[:, 0:256],
                        mm(w_sb[name][:, dc, cc * P : (cc + 1) * P]),
                        mm(st["xt"][:, dc, s0 : s0 + 256]),
                        start=(dc == 0),
                        stop=(dc == DB - 1),
                    )
                nc.vector.tensor_scalar_add(
                    dsttile[:, cc, blk * 512 + s0 : blk * 512 + s0 + 256],
                    ps[:, 0:256],
                    b_sb[name][:, cc : cc + 1],
                )

            def emit_pv(blk, t):
                st = bstate[("v", blk)]
                sc = blk * 4 + t
                ps = ps_wk.tile([P, 512], f32, tag="wk")
                for dc in range(DB):
                    nc.tensor.matmul(
                        ps[:, 0:C],
                        mm(st["xt"][:, dc, t * P : (t + 1) * P]),
                        mm(w_sb["v"][:, dc, :]),
                        start=(dc == 0),
                        stop=False,
                    )
                nc.tensor.matmul(
                    ps[:, 0:C],
                    mm(ones_row[:, :]),
                    mm(bv_row[:, :]),
                    start=False,
                    stop=True,
                )
                nc.vector.tensor_copy(
                    v1[:, sc, :, 0:HD],
                    ps[:, 0:C].rearrange("p (h e) -> p h e", h=NH),
                )

            def block_items(name, blk):
                # items tagged with a drain sub-key; for v the sub-key is
                # per-kt so AV forces drain only what they need
                items = []
                if name in ("q", "k"):
                    key = (name, blk)
                    for t in range(4):
                        for half in range(2):
                            items.append(
                                (key, 0.45, (lambda n, b, tt, hh: lambda: emit_tr(n, b, tt, hh))(name, blk, t, half))
                            )
                    for cc in range(CB):
                        for sh in range(2):
                            items.append(
                                (key, 0.9, (lambda n, b, c, s_: lambda: emit_pj(n, b, c, s_))(name, blk, cc, sh))
                            )
                else:
                    for t in range(4):
                        key = ("v", blk, t)
                        for half in range(2):
                            items.append(
                                (key, 0.45, (lambda b, tt, hh: lambda: emit_tr("v", b, tt, hh))(blk, t, half))
                            )
                        items.append(
                            (key, 1.0, (lambda b, tt: lambda: emit_pv(b, tt))(blk, t))
                        )
                return items

            # ---------------- prefix ----------------
            for (name, blk) in [(n, b) for n in ("q", "k", "v") for b in range(NBLK)]:
                bstate[(name, blk)] = {}
            emit_dma("k", 0)
            emit_w("k")
            emit_dma("q", 0)
            emit_w("q")
            emit_w("v")
            nc.gpsimd.dma_start(bv_row[:], bv[None, :])
            for name, ap in (("q", bq), ("k", bk)):
                t = const_pool.tile([P, CB], f32, tag=f"b_{name}", name=f"b_{name}")
                nc.sync.dma_start(t[:], ap.rearrange("(cc p) -> p cc", p=P))
                b_sb[name] = t
            # q1 before v0: q1 gates scores (slot 8) directly, while v0's AV
            # consumers tolerate lag through the pex pool
            if NQB > 1:
                emit_dma("q", 1)
            emit_dma("v", 0)
            for _, cost, fn in block_items("k", 0) + block_items("q", 0):
                fn()

            # ---------------- weave queue ----------------
            queue_blocks = [("v", 0)]
            queue_blocks += [("q", b) for b in range(1, NQB)]
            for b in range(1, NBLK):
                queue_blocks += [("k", b), ("v", b)]
            qitems = {key: block_items(*key) for key in queue_blocks}
            # DMA for block i leads by one queue position (v0 and q1 DMAs are
            # already emitted in the prefix)
            for i, key in enumerate(queue_blocks):
                if key in (("v", 0), ("q", 1)):
                    continue
                lead = queue_blocks[max(0, i - 1)]
                dma_fn = (lambda k: lambda: emit_dma(*k))(key)
                qitems[lead].insert(0, (("dma",) + key, 0.15, dma_fn))
            qlist = [
                (subkey, cost, fn)
                for key in queue_blocks
                for subkey, cost, fn in qitems[key]
            ]
            qpos = [0]  # next index into qlist
            total_cost = sum(c for _, c, _ in qlist)
            done_upto = {}
            for i, (key, _, _) in enumerate(qlist):
                done_upto[key] = i + 1  # drain-through index per block

            def drain_through(key):
                tgt = done_upto.get(key, 0)
                while qpos[0] < tgt:
                    _, _, fn = qlist[qpos[0]]
                    fn()
                    qpos[0] += 1

            cum = [0.0]
            # deadline-aware drain plan: piecewise-linear cum-cost targets so
            # forced drains never burst (bursts block scores in the PE FIFO
            # and starve the exp stream)
            block_deadline = {}
            for key in queue_blocks:
                name, b = key
                if name == "v":
                    block_deadline[key] = max(2, b * NU * KB - 1)
                elif name == "q":
                    block_deadline[key] = max(2, b * CB * KB - 1)
                else:  # k
                    block_deadline[key] = max(2, b * NU * KB - 3)
            plan = []  # (slot, cum_cost_required)
            run = 0.0
            for key in queue_blocks:
                run += sum(c for _, c, _ in qitems[key])
                plan.append((block_deadline[key], run))
            plan.sort()

            def plan_target(slot):
                prev_s, prev_c = 0, 0.0
                for ds, dc in plan:
                    if slot < ds:
                        return prev_c + (dc - prev_c) * (slot + 1 - prev_s) / max(
                            1, ds - prev_s
                        )
                    prev_s, prev_c = ds, dc
                return total_cost

            def budget_pop(slot, n_slots, drain_slots):
                tgt = max(
                    plan_target(slot),
                    total_cost * min(1.0, (slot + 1) / max(1, drain_slots)),
                )
                while qpos[0] < len(qlist) and cum[0] < tgt:
                    _, c, fn = qlist[qpos[0]]
                    fn()
                    qpos[0] += 1
                    cum[0] += c

            # ---------------- stream ----------------
            avpack = os.environ.get("MHA_AVPACK", "0") == "1"

            def emit_av(hp, av, pex, ktp, first, last):
                if avpack:
                    for head, half in ((0, 0), (1, 1), (0, 1), (1, 0)):
                        r0 = half * 64
                        nc.tensor.matmul(
                            av[head][0 : HD + 1, :],
                            mm(v1[r0 : r0 + 64, ktp, 2 * hp + head, :]),
                            mm(pex[r0 : r0 + 64, head * QB : (head + 1) * QB]),
                            start=(first and half == head),
                            stop=(last and half != head),
                        )
                    return
                for head in range(2):
                    nc.tensor.matmul(
                        av[head][0 : HD + 1, :],
                        mm(v1[:, ktp, 2 * hp + head, :]),
                        mm(pex[:, head * QB : (head + 1) * QB]),
                        start=first,
                        stop=last,
                    )

            def emit_tail_p1(u, av):
                # softmax-fold phase: DVE combines the last sweep's PSUM
                # partials with the SBUF accumulator; runs at the flush so the
                # av pool slots free up immediately
                osbs = []
                for head in range(2):
                    o_sb = o_pool.tile(
                        [HD + 1, QB], f32, tag="osb", name=f"osb{u}_{head}"
                    )
                    if NBLK > 1:
                        nc.vector.scalar_tensor_tensor(
                            o_sb[:],
                            av[head][0 : HD + 1, :],
                            1.0,
                            o_acc[:, u, head, :],
                            op0=mybir.AluOpType.mult,
                            op1=mybir.AluOpType.add,
                        )
                    else:
                        nc.vector.tensor_copy(o_sb[:], av[head][0 : HD + 1, :])
                    osbs.append(o_sb)
                return osbs

            def emit_tail_p2(u, osbs):
                # transpose+normalize phase: emitted two slots later so the PE
                # transposes never wait on the p1 DVE adds in the FIFO
                qb, hp = divmod(u, CB)
                out_sb = out_pool.tile(
                    [P, NJ, P], f32, tag="outsb", name=f"outsb{u}"
                )
                for head in range(2):
                    tp = ps_wk.tile([P, 512], f32, tag="wk", name=f"tp{u}_{head}")
                    tpv = tp.rearrange("p (i q) -> p i q", q=P)
                    for j in range(NJ):
                        nc.tensor.transpose(
                            tpv[:, j, 0 : HD + 1],
                            osbs[head][:, j * P : (j + 1) * P],
                            ident_f[: HD + 1, : HD + 1],
                        )
                    rsb = o_pool.tile([P, NJ], f32, tag="rsb", name=f"rsb{u}_{head}")
                    nc.vector.reciprocal(rsb[:], tpv[:, :, HD])
                    for j in range(NJ):
                        nc.vector.tensor_scalar_mul(
                            out_sb[:, j, head * HD : (head + 1) * HD],
                            tpv[:, j, 0:HD],
                            rsb[:, j : j + 1],
                        )
                q0r = qb * QB
                nc.sync.dma_start(
                    out[q0r : q0r + QB, hp * P : (hp + 1) * P].rearrange(
                        "(j p) c -> p j c", p=P
                    ),
                    out_sb[:],
                )

            n_slots = NBLK * NU * KB
            drain_slots = max(1, n_slots - NU * KB)
            pend = None  # (u, hp, av, pex, ktp, kb)
            tail2 = None  # (u, osbs) awaiting phase-2
            slot = 0
            for kb in range(NBLK):
                for u in range(NU):
                    qb, hp = divmod(u, CB)
                    drain_through(("k", kb))
                    drain_through(("q", qb))
                    av = None
                    q0r = qb * QB
                    pexs = {}
                    for j in range(KB):
                        kt = kb * KB + j
                        sc_ps = ps_sc.tile([P, 2 * QB], f32, tag="sc")
                        for head in range(2):
                            r0 = head * HD
                            nc.tensor.matmul(
                                sc_ps[:, head * QB : (head + 1) * QB],
                                mm(kT[r0 : r0 + HD, hp, kt * P : (kt + 1) * P]),
                                mm(qT[r0 : r0 + HD, hp, q0r : q0r + QB]),
                                start=True,
                                stop=True,
                            )
                        pex = pex_pool.tile([P, 2 * QB], dt_x, tag="pex")
                        nc.scalar.activation(
                            pex[:], sc_ps[:], AF.Exp, bias=0.0, scale=0.125
                        )
                        pexs[j] = pex
                        if j == 2 and tail2 is not None:
                            emit_tail_p2(*tail2)
                            tail2 = None
                        if j == 0:
                            if pend is not None:
                                pu, php, pav, ppex, pktp, pkb = pend
                                drain_through(("v", pktp // KB, pktp % KB))
                                emit_av(php, pav, ppex, pktp, False, True)
                                if pkb == NBLK - 1:
                                    # p1 must run before this visit's av tiles
                                    # reuse the pool slots
                                    tail2 = (pu, emit_tail_p1(pu, pav))
                                elif pkb == 0 and NBLK > 1:
                                    for h in range(2):
                                        nc.vector.tensor_copy(
                                            o_acc[:, pu, h, :],
                                            pav[h][0 : HD + 1, :],
                                        )
                                else:
                                    for h in range(2):
                                        nc.vector.scalar_tensor_tensor(
                                            o_acc[:, pu, h, :],
                                            pav[h][0 : HD + 1, :],
                                            1.0,
                                            o_acc[:, pu, h, :],
                                            op0=mybir.AluOpType.mult,
                                            op1=mybir.AluOpType.add,
                                        )
                                pend = None
                            av = {
                                h: ps_av.tile(
                                    [P, 512],
                                    f32,
                                    tag="av",
                                    name=f"av{kb}_{u}_{h}",
                                )
                                for h in range(2)
                            }
                        else:
                            ktp = kt - 1
                            drain_through(("v", ktp // KB, ktp % KB))
                            emit_av(hp, av, pexs[j - 1], ktp, j - 1 == 0, False)
                        budget_pop(slot, n_slots, drain_slots)
                        slot += 1
                    pend = (u, hp, av, pexs[KB - 1], kb * KB + KB - 1, kb)
            # final flush
            pu, php, pav, ppex, pktp, pkb = pend
            drain_through(("v", pktp // KB, pktp % KB))
            emit_av(php, pav, ppex, pktp, KB == 1, True)
            if tail2 is not None:
                emit_tail_p2(*tail2)
            emit_tail_p2(pu, emit_tail_p1(pu, pav))
    nc.compile()
    return nc


_CACHE = {}


def _get_nc(dt_mode: str):
    key = (dt_mode, os.environ.get("MHA_V2", "0"))
    if key not in _CACHE:
        if key[1] == "1":
            _CACHE[key] = build_nc(dt_mode)
        else:
            _CACHE[key] = build_nc_v3(dt_mode)
    return _CACHE[key]


def kernel(query, key, value, Wq, bq, Wk, bk, Wv, bv, **kwargs):
    _install_ntff_hook_shim()
    from concourse.bass_utils import run_bass_kernel_spmd

    dt_mode = os.environ.get("MHA_DT", "fp16")
    nc = _get_nc(dt_mode)

    query = np.asarray(query, dtype=np.float32)
    key = np.asarray(key, dtype=np.float32)
    value = np.asarray(value, dtype=np.float32)
    Wq = np.asarray(Wq, dtype=np.float32)
    Wk = np.asarray(Wk, dtype=np.float32)
    Wv = np.asarray(Wv, dtype=np.float32)
    bq = np.asarray(bq, dtype=np.float32)
    bk = np.asarray(bk, dtype=np.float32)
    bv = np.asarray(bv, dtype=np.float32)

    in_maps = []
    for c in range(N_CORES):
        b, g = divmod(c, GROUPS)
        cs = g * C
        in_maps.append(
            {
                "xq": np.ascontiguousarray(query[b]),
                "xk": np.ascontiguousarray(key[b]),
                "xv": np.ascontiguousarray(value[b]),
                "wq": np.ascontiguousarray(Wq[:, cs : cs + C]),
                "wk": np.ascontiguousarray(Wk[:, cs : cs + C]),
                "wv": np.ascontiguousarray(Wv[:, cs : cs + C]),
                "bq": np.ascontiguousarray(bq[cs : cs + C]),
                "bk": np.ascontiguousarray(bk[cs : cs + C]),
                "bv": np.ascontiguousarray(bv[cs : cs + C]),
            }
        )

    res = run_bass_kernel_spmd(
        nc, in_maps, core_ids=list(range(N_CORES)), **kwargs
    )
    outp = np.empty((B, S, D), dtype=np.float32)
    for c in range(N_CORES):
        b, g = divmod(c, GROUPS)
        outp[b, :, g * C : (g + 1) * C] = res.results[c]["out"]
    if kwargs:
        return outp, res
    return outp

